# revision 37
# baseline (speedup 1.0000x reference)
"""Bahdanau additive attention on TRN2, data-parallel over batch on 8 NeuronCores.

Reference computation (per batch b):
    pre[s, :]  = W1 @ hs[s, b, :] + b1 + W2 @ hidden[b, :] + b2      # (S, H)
    energy[s]  = v . tanh(pre[s, :])                                  # (S,)
    energy     = where(mask[s, b], energy, -1e10)
    attn       = softmax(energy over s)
    ctx[b, :]  = sum_s attn[s] * hs[s, b, :]                          # (H,)

Sparsity: masked positions get attn == 0 exactly, so the host packs only the
unmasked rows of hs per batch (~50% of S) into a fixed SP-wide layout, padded
with zero columns that the device masks out of the softmax.

Per-core layout (batch-sharded, BL=4 batches per core):
  - Sigma blocks are variable width [512, 512(, 128...)] chosen from the
    actual max unmasked count: wide matmuls amortize the ~35ns fixed
    per-matmul overhead (measured: N=384 -> 195ns, N=512 -> 240ns, and the
    overhead is NOT LDWEIGHTS -- repeating the same stationary doesn't help).
  - Every DMA'd operand is its own tile (w1 per m-chunk, hst per k-chunk,
    hsn per s-tile) so Tile's dependency tracking is per-chunk and the first
    matmul only gates on w1[m0] + hst[k0] instead of the whole 4.6MB.
  - PE warmup runs ~45 N=128 matmuls on memset garbage so HAM reaches
    K=8/8 before the first real matmul (no DMA dependency at all).
  - The block tail is split into three closures flushed at separate slots in
    later blocks' matmul streams: tp (pen matmul + mask + max + exp),
    tmid (PE transposes + attZ copies), post (context matmuls). This gives
    the ACT/DVE chain 2+ m-groups of latency headroom so the in-order PE
    queue never stalls on it (the baseline lost ~1.9us per block here).
    The merge is likewise split into merge_a (DVE chain + csum->SBUF
    evacuation) and merge_b (the weighted-merge matmuls + output DMA).
  - pen / merge matmuls take fp16 moving operands (f32r streams at 1/4 rate
    at full clock for narrow tiles; fp16 error on the energy is ~1e-3,
    far inside the 2e-2 gate).
"""

import sys
from contextlib import ExitStack

import numpy as np

if "/opt/trn_rl_repo" not in sys.path:
    sys.path.append("/opt/trn_rl_repo")

import concourse.bass as bass
import concourse.bacc as bacc
import concourse.mybir as mybir
import concourse.tile as tile
from concourse import bass_utils

S, B, H = 2048, 32, 1024
NCORES = 8
BL = B // NCORES  # local batches per core
HK = H // 128     # 128-partition chunks of H

F32 = mybir.dt.float32
F32R = mybir.dt.float32r
U8 = mybir.dt.uint8
FP16 = mybir.dt.float16
AF = mybir.ActivationFunctionType
AX = mybir.AxisListType
OP = mybir.AluOpType

_CACHE = {}


def pick_blocks(masks):
    """Sigma-block widths [512]*q + [128*r]: minimal 128-multiple >= maxn."""
    maxn = int(np.asarray(masks).sum(axis=0).max())
    maxn = max(maxn, 128)
    q, rem = divmod(maxn, 512)
    blocks = [512] * q
    if rem:
        blocks.append(128 * ((rem + 127) // 128))
    return tuple(blocks)


def _emit(tc, aps, blocks):
    nc = tc.nc
    ctx = aps["ctx_stack"]
    NSIG = len(blocks)
    SP = sum(blocks)
    OFF = [sum(blocks[:c]) for c in range(NSIG)]
    TPB = [w // 128 for w in blocks]
    GBASE = [OFF[c] // 128 for c in range(NSIG)]
    NT = SP // 128
    hst, hsn, w1r, w2t, bvt, hidr, masku, ctx_out = (
        aps["hst"], aps["hsn"], aps["w1r"], aps["w2t"],
        aps["bvt"], aps["hidr"], aps["masku"], aps["ctx"],
    )

    def pool(name, bufs, space="SBUF"):
        return ctx.enter_context(tc.tile_pool(name=name, bufs=bufs, space=space))

    big = SP > 1152  # rare dense-mask case: trade overlap for SBUF headroom
    p_w1 = pool("w1", 1)
    p_w2 = pool("w2", 1)
    p_hst = pool("hst", 2)     # half-batch tiles, 2 batches in flight
    p_hsn = pool("hsn", 1 if SP > 1280 else 2)
    p_small = pool("small", 1)
    p_pre0 = pool("pre0", 1)
    p_tanh = pool("tanh", 3)
    p_acc = pool("acc", 6)
    p_em = pool("em", 3)
    p_ctxs = pool("ctxs", 2)
    p_attZ = pool("attZ", 2)
    p_sc = pool("sc", 4)
    p_mx = pool("mx", 8)

    pp_pre = pool("ppre", 5, space="PSUM")
    pp_misc = pool("pmisc", 1, space="PSUM")
    pp_ctx = pool("pctx", 2, space="PSUM")

    # ---------------- setup DMAs ----------------
    # Host relays every big operand so one DMA moves 9-18KB per partition
    # (fat descriptors ~300GB/s; the naive per-row 2KB descriptors cap a
    # queue row at ~60-100GB/s).  hst is split in two half-batch tiles so
    # block (0,0)'s first k-chunks gate on only half the bytes.
    MA = 5  # phase-A m-count for block (0,0)
    w1_t = [
        p_w1.tile([128, H], FP16, tag=f"w1_{m}", name=f"w1_{m}", bufs=1)
        for m in range(HK)
    ]
    w2_sb = p_w2.tile([128, HK * H], FP16, tag="w2", name="w2_sb")
    hstb_t = {}

    HA = HK // 2

    def hst_dma(b, eng=None):
        """Fat half-batch transfers (steady-state batches), split over two
        queue rows (each row gets ~1/3 of aggregate DMA bandwidth)."""
        ta = p_hst.tile([128, HA * SP], FP16, tag="hstr_a", name=f"hst_{b}_a")
        tb = p_hst.tile([128, (HK - HA) * SP], FP16, tag="hstr_b", name=f"hst_{b}_b")
        (eng or nc.gpsimd).dma_start(ta[:], hst[b, :, :HA * SP])
        (eng or nc.scalar).dma_start(tb[:], hst[b, :, HA * SP:])
        hstb_t[b] = [
            (ta[:, SP * k:SP * (k + 1)] if k < HA
             else tb[:, SP * (k - HA):SP * (k - HA + 1)])
            for k in range(HK)
        ]

    hsn_t = {}

    def hsn_dma(b, eng=None):
        h = p_hsn.tile([128, NT * H], FP16, tag="hsnr", name=f"hsn_{b}")
        (eng or nc.sync).dma_start(h[:], hsn[b, :, :])
        hsn_t[b] = [h[:, H * t:H * (t + 1)] for t in range(NT)]

    # warmup source: memset garbage, no DMA dependency.  On DVE -- the
    # gpsimd engine queue must start with its dma_starts (SWDGE) or the
    # first gpsimd-row DMA lands several us late.
    wsrc = p_small.tile([128, 128], FP16, tag="wsrc")
    nc.vector.memset(wsrc[:], 0.5)
    ident = p_small.tile([1, 1], F32, tag="ident")
    nc.vector.memset(ident[:], 1.0)
    ones_h = p_small.tile([128, 1], FP16, tag="ones_h")
    nc.vector.memset(ones_h[:], 1.0)

    # Head-priority DMAs.  Block (0,0) phase A consumes w1[m] x hst0[k]
    # progressively, so both are per-chunk tiles emitted in consumption
    # order, interleaved across the three queue rows (each row gets ~1/3 of
    # the ~300GB/s aggregate while all are busy).
    def w1_dma(m):
        nc.sync.dma_start(w1_t[m][:], w1r[:, H * m:H * (m + 1)])

    if big:
        w1_dma(0)
        hst_dma(0)  # fat halves on gpsimd+scalar; phase A paces off arrival
        for m in range(1, HK):
            w1_dma(m)
    else:
        tiles0 = [
            p_hst.tile([128, SP], FP16, tag=f"h0k{k}", name=f"hst_0_{k}", bufs=1)
            for k in range(HK)
        ]
        hstb_t[0] = [t[:] for t in tiles0]

        def hst0_dma(k, eng):
            eng.dma_start(tiles0[k][:], hst[0, :, SP * k:SP * (k + 1)])

        w1_dma(0)
        hst0_dma(0, nc.scalar)
        hst0_dma(3, nc.gpsimd)
        w1_dma(1)
        hst0_dma(1, nc.scalar)
        hst0_dma(4, nc.gpsimd)
        w1_dma(2)
        w1_dma(3)
        hst0_dma(2, nc.scalar)
        hst0_dma(5, nc.gpsimd)
        w1_dma(4)
        w1_dma(5)
        hst0_dma(6, nc.gpsimd)
        w1_dma(6)
        w1_dma(7)
        hst0_dma(7, nc.gpsimd)

    bvt_sb = p_small.tile([128, 3 * HK + 5], F32R, tag="bvt")
    nc.scalar.dma_start(bvt_sb[:], bvt[:])
    b1_sb = bvt_sb[:, 0:HK].bitcast(F32)
    b2_sb = bvt_sb[:, HK:2 * HK].bitcast(F32)
    vt_sb = bvt_sb[:, 2 * HK:3 * HK].bitcast(F32)
    eye4 = bvt_sb[0:4, 3 * HK:3 * HK + 4].bitcast(F32)
    hid_sb = p_small.tile([128, BL * HK], FP16, tag="hidr")
    nc.scalar.dma_start(hid_sb[:], hidr[:])
    mask_all = p_small.tile([1, BL * SP], U8, tag="mask")
    nc.scalar.dma_start(mask_all[:], masku[:])
    # W2 (fat, one transfer) behind hst0-half on scalar; hsn0 behind w1 on sync
    nc.scalar.dma_start(w2_sb[:], w2t[:])
    hsn_dma(0, eng=nc.sync)

    qt_sb = p_small.tile([128, BL * HK], F32, tag="qt")

    # PE clock warmup on garbage: dense N=128 matmuls until HAM hits K=8/8
    warm = pp_misc.tile([128, 128], F32, tag="ptr", name="warm")
    for _ in range(45):
        nc.tensor.matmul(warm[:], lhsT=wsrc[:], rhs=wsrc[:], start=True, stop=True)

    # ------------- pending-closure FIFO -------------
    pending = []

    def flush_one():
        if pending:
            pending.pop(0)()

    pre0_t = {}
    acc_t = {}

    def _act_chain_m(b, c, m, src, W):
        th = p_tanh.tile([128, W], F32, tag="tanh", name=f"th_{b}_{c}_{m}")
        nc.scalar.activation(
            th[:], src, AF.Tanh,
            bias=qt_sb[:, BL * m + b:BL * m + b + 1], scale=1.0,
        )
        if m == 0:
            acc = p_acc.tile([128, W], F32, tag="acc", name=f"acc_{b}_{c}")
            acc_t[(b, c)] = acc
            nc.vector.tensor_scalar_mul(acc[:], th[:], vt_sb[:, 0:1])
        elif m < HK - 1:
            acc = acc_t[(b, c)]
            nc.vector.scalar_tensor_tensor(
                acc[:], th[:], vt_sb[:, m:m + 1], acc[:], op0=OP.mult, op1=OP.add,
            )
        else:
            acc = acc_t.pop((b, c))
            acc_r = p_acc.tile([128, W], FP16, tag="accr", name=f"accr_{b}_{c}")
            acc_t[(b, c)] = acc_r
            nc.vector.scalar_tensor_tensor(
                acc_r[:], th[:], vt_sb[:, m:m + 1], acc[:], op0=OP.mult, op1=OP.add,
            )

    def p1_block00():
        """Block (0,0), deferred acts, DMA-paced: k-outer over 5 PSUM banks so
        the PE consumes hst chunks at arrival pace with no >1us idle windows
        (idle windows re-throttle HAM to 1.2GHz), then k-inner for the rest."""
        W, off = blocks[0], OFF[0]
        pre0 = p_pre0.tile([128, HK * W], F32, tag="pre0", name="pre0_0_0")
        pre0_t[(0, 0)] = pre0
        hstb = hstb_t[0]
        pp = [
            pp_pre.tile([128, W], F32, tag="ppre", name=f"pp0_{m}")
            for m in range(MA)
        ]
        for k in range(HK):
            if k:
                # filler matmuls on garbage: the k-step is DMA-arrival-bound,
                # so these are free and keep HAM from re-throttling the PE
                # during the chunk gaps (early steps wait longest)
                for _ in range(12):
                    nc.tensor.matmul(warm[:], lhsT=wsrc[:], rhs=wsrc[:],
                                     start=True, stop=True)
            for m in range(MA):
                nc.tensor.matmul(
                    pp[m][:],
                    lhsT=w1_t[m][:, 128 * k:128 * (k + 1)],
                    rhs=hstb[k][:, off:off + W],
                    start=(k == 0), stop=(k == HK - 1),
                )
        for m in range(MA):
            nc.vector.tensor_copy(pre0[:, W * m:W * (m + 1)], pp[m][:])
        for m in range(MA, HK):
            ppre = pp_pre.tile([128, W], F32, tag="ppre", name=f"ppre_0_0_{m}")
            for k in range(HK):
                nc.tensor.matmul(
                    ppre[:],
                    lhsT=w1_t[m][:, 128 * k:128 * (k + 1)],
                    rhs=hstb[k][:, off:off + W],
                    start=(k == 0), stop=(k == HK - 1),
                )
            nc.vector.tensor_copy(pre0[:, W * m:W * (m + 1)], ppre[:])

    def p1_block(b, c, all_slots=False):
        W, off = blocks[c], OFF[c]
        if c == min(1, NSIG - 1) and b + 1 < BL:
            hst_dma(b + 1)
        slots = set(range(HK)) if all_slots else {1, 3, 5, 7}
        if NSIG == 2 and not all_slots:
            slots |= {2, 6}
        hstb = hstb_t[b]
        for m in range(HK):
            if m in slots:
                flush_one()
            ppre = pp_pre.tile([128, W], F32, tag="ppre", name=f"ppre_{b}_{c}_{m}")
            for k in range(HK):
                nc.tensor.matmul(
                    ppre[:],
                    lhsT=w1_t[m][:, 128 * k:128 * (k + 1)],
                    rhs=hstb[k][:, off:off + W],
                    start=(k == 0), stop=(k == HK - 1),
                )
            _act_chain_m(b, c, m, ppre[:], W)

    def p1_acts(b, c):
        """Deferred activation chains for the SBUF-buffered batch-0 blocks."""
        W = blocks[c]
        pre0 = pre0_t.pop((b, c))
        for m in range(HK):
            _act_chain_m(b, c, m, pre0[:, W * m:W * (m + 1)], W)

    # ------------- per-block tail: tp -> tmid -> post -------------
    mx_t, zf_t, em_t, attZ_t, csum_t = {}, {}, {}, {}, {}

    def tail_tp(b, c):
        W, off = blocks[c], OFF[c]
        acc_r = acc_t.pop((b, c))
        pen = pp_misc.tile([1, W], F32, tag="ptr", name=f"pen_{b}_{c}")
        nc.tensor.matmul(pen[:], lhsT=ones_h[:], rhs=acc_r[:], start=True, stop=True)
        if c == 0:
            mx_t[b] = p_mx.tile([1, NSIG], F32, tag="mx", name=f"mx{b}")
            zf_t[b] = p_mx.tile([1, NSIG], F32, tag="zf", name=f"zf{b}")
        em = p_em.tile([1, W], F32, tag="em", name=f"em_{b}_{c}")
        nc.vector.scalar_tensor_tensor(
            em[:], mask_all[:, b * SP + off:b * SP + off + W], -1e10, pen[:],
            op0=OP.mult, op1=OP.add,
        )
        em_t[(b, c)] = em
        nc.vector.reduce_max(mx_t[b][:, c:c + 1], em[:], axis=AX.X)
        nmx = p_sc.tile([1, 1], F32, tag="nmx", name=f"nmx_{b}_{c}")
        nc.vector.tensor_scalar_mul(nmx[:], mx_t[b][:, c:c + 1], -1.0)
        nc.scalar.activation(
            em[:], em[:], AF.Exp, bias=nmx[:], scale=1.0,
            accum_out=zf_t[b][:, c:c + 1],
        )

    def tail_tmid(b, c):
        T = TPB[c]
        em = em_t.pop((b, c))
        if c == 0:
            attZ = p_attZ.tile([128, NSIG * NT], FP16, tag="attZ", name=f"attZ{b}")
            nc.vector.memset(attZ[:], 0.0)
            attZ_t[b] = attZ
        attZ = attZ_t[b]
        ptr = pp_misc.tile([128, T], F32, tag="ptr", name=f"ptr_{b}_{c}")
        for j in range(T):
            nc.tensor.transpose(ptr[:, j:j + 1], em[:, 128 * j:128 * (j + 1)], ident[:])
        for j in range(T):
            t = GBASE[c] + j
            nc.vector.tensor_copy(attZ[:, NSIG * t + c:NSIG * t + c + 1], ptr[:, j:j + 1])

    def tail_post(b, c):
        T = TPB[c]
        attZ = attZ_t[b]
        if c == 0:
            csum_t[b] = [
                pp_ctx.tile([NSIG, 512], F32, tag="pctx", name=f"csum_{b}_{n}")
                for n in range(2)
            ]
        csum = csum_t[b]
        tiles = hsn_t[b]
        for j in range(T):
            t = GBASE[c] + j
            for n in range(2):
                nc.tensor.matmul(
                    csum[n][:],
                    lhsT=attZ[:, NSIG * t:NSIG * (t + 1)],
                    rhs=tiles[t][:, 512 * n:512 * (n + 1)],
                    start=(c == 0 and j == 0), stop=(c == NSIG - 1 and j == T - 1),
                )

    # ------------- q phase: qT[h_out, b] = W2 @ hidden + b1 + b2 ------------
    def q_phase():
        qn_sb = p_small.tile([BL, H], F32, tag="qnat")
        for n in range(2):
            pq = pp_ctx.tile([BL, 512], F32, tag="pctx", name=f"pq{n}")
            for k in range(HK):
                nc.tensor.matmul(
                    pq[:],
                    lhsT=hid_sb[:, BL * k:BL * (k + 1)],
                    rhs=w2_sb[:, H * k + 512 * n:H * k + 512 * (n + 1)],
                    start=(k == 0), stop=(k == HK - 1),
                )
            nc.vector.tensor_copy(qn_sb[:, 512 * n:512 * (n + 1)], pq[:])
        ptrq = pp_misc.tile([128, BL * HK], F32, tag="ptr", name="ptrq")
        for m in range(HK):
            nc.tensor.transpose(
                ptrq[:, BL * m:BL * (m + 1)], qn_sb[:, 128 * m:128 * (m + 1)], eye4
            )
        for m in range(HK):
            nc.vector.tensor_scalar_add(
                qt_sb[:, BL * m:BL * (m + 1)], ptrq[:, BL * m:BL * (m + 1)], b1_sb[:, m:m + 1]
            )
            nc.vector.tensor_scalar_add(
                qt_sb[:, BL * m:BL * (m + 1)], qt_sb[:, BL * m:BL * (m + 1)], b2_sb[:, m:m + 1]
            )

    # ------------- merge: merge_a (DVE chain + evac) / merge_b (matmuls) ----
    wrz_t, csr_t = {}, {}

    def merge_a(b):
        mx = mx_t.pop(b)
        zf = zf_t.pop(b)
        attZ_t.pop(b)
        csum = csum_t.pop(b)
        negM = p_sc.tile([1, 1], F32, tag="negM", name=f"negM{b}")
        nc.vector.reduce_max(negM[:], mx[:], axis=AX.X, negate=True)
        wt = p_mx.tile([1, NSIG], F32, tag="wt", name=f"wt{b}")
        nc.scalar.activation(wt[:], mx[:], AF.Exp, bias=negM[:], scale=1.0)
        zw = p_mx.tile([1, NSIG], F32, tag="zw", name=f"zw{b}")
        nc.vector.tensor_mul(zw[:], wt[:], zf[:])
        zs = p_sc.tile([1, 1], F32, tag="zs", name=f"zs{b}")
        nc.vector.reduce_sum(zs[:], zw[:], axis=AX.X)
        rz = p_sc.tile([1, 1], F32, tag="rz", name=f"rz{b}")
        nc.vector.reciprocal(rz[:], zs[:])
        wrz = p_mx.tile([1, NSIG], F32, tag="wrz", name=f"wrz{b}")
        nc.vector.tensor_scalar_mul(wrz[:], wt[:], rz[:])
        ptrw = pp_misc.tile([NSIG, 1], F32, tag="ptr", name=f"ptrw{b}")
        nc.tensor.transpose(ptrw[:], wrz[:], ident[:])
        wrz_h = p_mx.tile([NSIG, 1], FP16, tag="wrz_h", name=f"wrz_h{b}")
        nc.vector.tensor_copy(wrz_h[:], ptrw[:])
        wrz_t[b] = wrz_h
        csr = p_ctxs.tile([NSIG, H], FP16, tag="csr", name=f"csr{b}")
        for n in range(2):
            nc.vector.tensor_copy(csr[:, 512 * n:512 * (n + 1)], csum[n][:])
        csr_t[b] = csr

    def merge_b(b):
        wrz_h = wrz_t.pop(b)
        csr = csr_t.pop(b)
        cs = p_ctxs.tile([1, H], F32, tag="cs", name=f"cs{b}")
        for n in range(2):
            mo = pp_misc.tile([1, 512], F32, tag="ptr", name=f"mo_{b}_{n}")
            nc.tensor.matmul(
                mo[:], lhsT=wrz_h[:], rhs=csr[:, 512 * n:512 * (n + 1)],
                start=True, stop=True,
            )
            # n0 evacuates via DVE so the ptr slot frees fast for mo n1;
            # n1 via ACT so the two copies run on different engines
            eng = nc.vector.tensor_copy if n == 0 else (
                lambda o, i: nc.scalar.activation(o, i, AF.Copy))
            eng(cs[:, 512 * n:512 * (n + 1)], mo[:])
            nc.sync.dma_start(
                ctx_out[b:b + 1, 512 * n:512 * (n + 1)], cs[:, 512 * n:512 * (n + 1)]
            )

    def append_tail(b, c):
        pending.append(lambda: tail_tp(b, c))
        pending.append(lambda: tail_tmid(b, c))
        pending.append(lambda: tail_post(b, c))
        if c == 0 and b + 1 < BL:
            # hsn prefetch goes through the FIFO so it is traced AFTER the
            # previous batch's context matmuls released the hsn slots.
            pending.append(lambda: hsn_dma(b + 1))

    # ------------- schedule -------------
    p1_block00()
    q_phase()
    if NSIG == 1:
        hst_dma(1)
    pending.append(lambda: p1_acts(0, 0))
    # spacers: tail_tp(0,0)'s pen matmul needs the deferred ACT chain
    # (~6us) to have finished by its flush slot
    pending.append(lambda: None)
    pending.append(lambda: None)
    append_tail(0, 0)
    for c in range(1, NSIG):
        p1_block(0, c)
        append_tail(0, c)
    pending.append(lambda: merge_a(0))
    pending.append(lambda: merge_b(0))
    for b in range(1, BL):
        for c in range(NSIG):
            p1_block(b, c, all_slots=(b == BL - 1))
            append_tail(b, c)
        pending.append(lambda bb=b: merge_a(bb))
        pending.append(lambda bb=b: merge_b(bb))
    while pending:
        flush_one()


def build_program(blocks):
    blocks = tuple(blocks)
    key = ("nc", blocks)
    if key in _CACHE:
        return _CACHE[key]
    SP = sum(blocks)
    nc = bacc.Bacc("TRN2", target_bir_lowering=False, debug=False, enable_asserts=False)
    NT = SP // 128
    aps = {
        "hst": nc.dram_tensor("hst", (BL, 128, HK * SP), FP16, kind="ExternalInput").ap(),
        "hsn": nc.dram_tensor("hsn", (BL, 128, NT * H), FP16, kind="ExternalInput").ap(),
        "w1r": nc.dram_tensor("w1r", (128, HK * H), FP16, kind="ExternalInput").ap(),
        "w2t": nc.dram_tensor("w2t", (128, HK * H), FP16, kind="ExternalInput").ap(),
        "bvt": nc.dram_tensor("bvt", (128, 3 * HK + 5), F32R, kind="ExternalInput").ap(),
        "hidr": nc.dram_tensor("hidr", (128, BL * HK), FP16, kind="ExternalInput").ap(),
        "masku": nc.dram_tensor("masku", (1, BL * SP), U8, kind="ExternalInput").ap(),
        "ctx": nc.dram_tensor("ctx", (BL, H), F32, kind="ExternalOutput").ap(),
    }
    with tile.TileContext(nc) as tc:
        with ExitStack() as stack:
            aps["ctx_stack"] = stack
            _emit(tc, aps, blocks)
    nc.compile()
    _CACHE[key] = nc
    return nc


def prep_in_maps(inputs, blocks):
    SP = sum(blocks)
    hidden = np.ascontiguousarray(np.asarray(inputs["hidden"], dtype=np.float32))
    hs = np.asarray(inputs["hidden_sequence"], dtype=np.float32)
    masks = np.asarray(inputs["input_masks"]).astype(bool)
    w1t = np.asarray(inputs["W1"], dtype=np.float32).T  # (hin, hout)
    # m-major relayout: w1r[:, 1024*m + 128*k : +128] = W1T[128k:128(k+1), 128m:128(m+1)]
    w1r = np.ascontiguousarray(
        w1t.reshape(HK, 128, HK, 128).transpose(1, 2, 0, 3).reshape(128, HK * H)
        .astype(np.float16)
    )
    # W2^T relaid to (128, HK*H): row p, block k = W2T[128k+p, :]
    w2tt = np.asarray(inputs["W2"], dtype=np.float32).T.astype(np.float16)
    w2t = np.ascontiguousarray(
        w2tt.reshape(HK, 128, H).transpose(1, 0, 2).reshape(128, HK * H))
    b1 = np.asarray(inputs["b1"], dtype=np.float32)
    b2 = np.asarray(inputs["b2"], dtype=np.float32)
    v = np.asarray(inputs["v"], dtype=np.float32)
    ey = np.zeros((128, 4), dtype=np.float32)
    ey[0:4, 0:4] = np.eye(4, dtype=np.float32)
    ones = np.ones((128, 1), dtype=np.float32)
    bvt_base = np.concatenate(
        [b1.reshape(HK, 128).T, b2.reshape(HK, 128).T, v.reshape(HK, 128).T, ey, ones],
        axis=1,
    )
    in_maps = []
    for ci in range(NCORES):
        g = slice(BL * ci, BL * (ci + 1))
        hg = hidden[0, g, :]  # (BL, H)
        hidr = np.ascontiguousarray(
            hg.T.reshape(HK, 128, BL).transpose(1, 0, 2).reshape(128, HK * BL).astype(np.float16)
        )
        NT = SP // 128
        hst_p = np.zeros((BL, H, SP), dtype=np.float16)
        hsn_p = np.zeros((BL, SP, H), dtype=np.float16)
        maskp = np.ones((BL, SP), dtype=np.uint8)  # 1 = padded (masked out)
        for b in range(BL):
            gb = BL * ci + b
            idx = np.nonzero(masks[:, gb])[0]
            n = idx.shape[0]
            sel = hs[idx, gb, :].astype(np.float16)  # (n, H)
            hst_p[b, :, :n] = sel.T
            hsn_p[b, :n, :] = sel
            maskp[b, :n] = 0
        # fat-descriptor relayouts: (BL,128,HK*SP) and (BL,128,NT*H)
        hst_r = np.ascontiguousarray(
            hst_p.reshape(BL, HK, 128, SP).transpose(0, 2, 1, 3)
            .reshape(BL, 128, HK * SP))
        hsn_r = np.ascontiguousarray(
            hsn_p.reshape(BL, NT, 128, H).transpose(0, 2, 1, 3)
            .reshape(BL, 128, NT * H))
        in_maps.append({
            "hst": hst_r,
            "hsn": hsn_r,
            "w1r": w1r,
            "w2t": w2t,
            "bvt": np.ascontiguousarray(bvt_base),
            "hidr": hidr,
            "masku": maskp.reshape(1, BL * SP),
        })
    return in_maps


def kernel(**inputs):
    blocks = pick_blocks(inputs["input_masks"])
    nc = build_program(blocks)
    in_maps = prep_in_maps(inputs, blocks)
    res = bass_utils.run_bass_kernel_spmd(nc, in_maps, list(range(NCORES)))
    out = np.concatenate([res.results[i]["ctx"] for i in range(NCORES)], axis=0)
    return out[None].astype(np.float32)


if __name__ == "__main__":
    build_program((512, 512, 128))
    print("program built OK")


# revision 53
# speedup vs baseline: 1.0986x; 1.0986x over previous
"""Bahdanau additive attention on TRN2, data-parallel over batch on 8 NeuronCores.

Reference computation (per batch b):
    pre[s, :]  = W1 @ hs[s, b, :] + b1 + W2 @ hidden[b, :] + b2      # (S, H)
    energy[s]  = v . tanh(pre[s, :])                                  # (S,)
    energy     = where(mask[s, b], energy, -1e10)
    attn       = softmax(energy over s)
    ctx[b, :]  = sum_s attn[s] * hs[s, b, :]                          # (H,)

Sparsity: masked positions get attn == 0 exactly, so the host packs only the
unmasked rows of hs per batch (~50% of S) into a fixed SP-wide layout, padded
with zero columns that the device masks out of the softmax.

Per-core layout (batch-sharded, BL=4 batches per core):
  - Sigma blocks are variable width [512, 512(, 128...)] chosen from the
    actual max unmasked count: wide matmuls amortize the ~35ns fixed
    per-matmul overhead (measured: N=384 -> 195ns, N=512 -> 240ns, and the
    overhead is NOT LDWEIGHTS -- repeating the same stationary doesn't help).
  - Every DMA'd operand is its own tile (w1 per m-chunk, hst per k-chunk,
    hsn per s-tile) so Tile's dependency tracking is per-chunk and the first
    matmul only gates on w1[m0] + hst[k0] instead of the whole 4.6MB.
  - PE warmup runs ~45 N=128 matmuls on memset garbage so HAM reaches
    K=8/8 before the first real matmul (no DMA dependency at all).
  - The block tail is split into three closures flushed at separate slots in
    later blocks' matmul streams: tp (pen matmul + mask + max + exp),
    tmid (PE transposes + attZ copies), post (context matmuls). This gives
    the ACT/DVE chain 2+ m-groups of latency headroom so the in-order PE
    queue never stalls on it (the baseline lost ~1.9us per block here).
    The merge is likewise split into merge_a (DVE chain + csum->SBUF
    evacuation) and merge_b (the weighted-merge matmuls + output DMA).
  - pen / merge matmuls take fp16 moving operands (f32r streams at 1/4 rate
    at full clock for narrow tiles; fp16 error on the energy is ~1e-3,
    far inside the 2e-2 gate).
"""

import sys
from contextlib import ExitStack

import numpy as np

if "/opt/trn_rl_repo" not in sys.path:
    sys.path.append("/opt/trn_rl_repo")

import concourse.bass as bass
import concourse.bacc as bacc
import concourse.mybir as mybir
import concourse.tile as tile
from concourse import bass_utils

S, B, H = 2048, 32, 1024
NCORES = 8
BL = B // NCORES  # local batches per core
HK = H // 128     # 128-partition chunks of H

F32 = mybir.dt.float32
F32R = mybir.dt.float32r
U8 = mybir.dt.uint8
FP16 = mybir.dt.float16
AF = mybir.ActivationFunctionType
AX = mybir.AxisListType
OP = mybir.AluOpType

_CACHE = {}


def pick_blocks(masks):
    """Sigma-block widths [512]*q + [128*r]: minimal 128-multiple >= maxn."""
    maxn = int(np.asarray(masks).sum(axis=0).max())
    maxn = max(maxn, 128)
    q, rem = divmod(maxn, 512)
    blocks = [512] * q
    if rem:
        blocks.append(128 * ((rem + 127) // 128))
    return tuple(blocks)


def _emit(tc, aps, blocks):
    nc = tc.nc
    ctx = aps["ctx_stack"]
    NSIG = len(blocks)
    SP = sum(blocks)
    OFF = [sum(blocks[:c]) for c in range(NSIG)]
    TPB = [w // 128 for w in blocks]
    GBASE = [OFF[c] // 128 for c in range(NSIG)]
    NT = SP // 128
    hst, hsn, w1r, qvt, masku, ctx_out = (
        aps["hst"], aps["hsn"], aps["w1r"],
        aps["qvt"], aps["masku"], aps["ctx"],
    )

    def pool(name, bufs, space="SBUF"):
        return ctx.enter_context(tc.tile_pool(name=name, bufs=bufs, space=space))

    big = SP > 1152  # rare dense-mask case: trade overlap for SBUF headroom
    p_w1 = pool("w1", 1)
    p_hst = pool("hst", 2)     # half-batch tiles, 2 batches in flight
    p_hsn = pool("hsn", 1 if SP > 1280 else 2)
    p_small = pool("small", 1)
    p_tanh = pool("tanh", 3)
    p_acc = pool("acc", 6)
    p_em = pool("em", 3)
    p_ctxs = pool("ctxs", 2)
    p_attZ = pool("attZ", 2)
    p_sc = pool("sc", 4)
    p_mx = pool("mx", 8)

    pp_pre = pool("ppre", 5, space="PSUM")
    pp_misc = pool("pmisc", 1, space="PSUM")
    pp_ctx = pool("pctx", 2, space="PSUM")

    # ---------------- setup DMAs ----------------
    # Host relays every big operand so one DMA moves 9-18KB per partition
    # (fat descriptors ~300GB/s; the naive per-row 2KB descriptors cap a
    # queue row at ~60-100GB/s).  hst is split in two half-batch tiles so
    # block (0,0)'s first k-chunks gate on only half the bytes.
    MA = 5  # phase-A m-count for block (0,0)
    w1_t = [
        p_w1.tile([128, H], FP16, tag=f"w1_{m}", name=f"w1_{m}", bufs=1)
        for m in range(HK)
    ]
    hstb_t = {}

    HA = HK // 2

    def hst_dma(b, eng=None):
        """Fat half-batch transfers (steady-state batches), split over two
        queue rows (each row gets ~1/3 of aggregate DMA bandwidth)."""
        ta = p_hst.tile([128, HA * SP], FP16, tag="hstr_a", name=f"hst_{b}_a")
        tb = p_hst.tile([128, (HK - HA) * SP], FP16, tag="hstr_b", name=f"hst_{b}_b")
        (eng or nc.gpsimd).dma_start(ta[:], hst[b, :, :HA * SP])
        (eng or nc.scalar).dma_start(tb[:], hst[b, :, HA * SP:])
        hstb_t[b] = [
            (ta[:, SP * k:SP * (k + 1)] if k < HA
             else tb[:, SP * (k - HA):SP * (k - HA + 1)])
            for k in range(HK)
        ]

    hsn_t = {}

    def hsn_dma(b, eng=None):
        h = p_hsn.tile([128, NT * H], FP16, tag="hsnr", name=f"hsn_{b}")
        (eng or nc.sync).dma_start(h[:], hsn[b, :, :])
        hsn_t[b] = [h[:, H * t:H * (t + 1)] for t in range(NT)]

    # warmup source: memset garbage, no DMA dependency.  On DVE -- the
    # gpsimd engine queue must start with its dma_starts (SWDGE) or the
    # first gpsimd-row DMA lands several us late.
    wsrc = p_small.tile([128, 128], FP16, tag="wsrc")
    nc.vector.memset(wsrc[:], 0.5)
    ident = p_small.tile([1, 1], F32, tag="ident")
    nc.vector.memset(ident[:], 1.0)
    ones_h = p_small.tile([128, 1], FP16, tag="ones_h")
    nc.vector.memset(ones_h[:], 1.0)

    # Head-priority DMAs.  Block (0,0) phase A consumes w1[m] x hst0[k]
    # progressively, so both are per-chunk tiles emitted in consumption
    # order, interleaved across the three queue rows (each row gets ~1/3 of
    # the ~300GB/s aggregate while all are busy).
    def w1_dma(m):
        nc.sync.dma_start(w1_t[m][:], w1r[:, H * m:H * (m + 1)])

    if big:
        w1_dma(0)
        hst_dma(0)  # fat halves on gpsimd+scalar; phase A paces off arrival
        for m in range(1, HK):
            w1_dma(m)
    else:
        tiles0 = [
            p_hst.tile([128, SP], FP16, tag=f"h0k{k}", name=f"hst_0_{k}", bufs=1)
            for k in range(HK)
        ]
        hstb_t[0] = [t[:] for t in tiles0]

        def hst0_dma(k, eng):
            eng.dma_start(tiles0[k][:], hst[0, :, SP * k:SP * (k + 1)])

        w1_dma(0)
        hst0_dma(0, nc.scalar)
        hst0_dma(3, nc.gpsimd)
        w1_dma(1)
        hst0_dma(1, nc.scalar)
        hst0_dma(4, nc.gpsimd)
        w1_dma(2)
        w1_dma(3)
        hst0_dma(2, nc.scalar)
        hst0_dma(5, nc.gpsimd)
        w1_dma(4)
        w1_dma(5)
        hst0_dma(6, nc.gpsimd)
        w1_dma(6)
        w1_dma(7)
        hst0_dma(7, nc.gpsimd)

    # qvt = [qt | vt] precomputed on host (q = W2 @ hidden + b1 + b2 is only
    # 0.004% of total FLOPs; computing it host-side removes the 2MB W2 DMA
    # from the critical window and the whole batch-0 activation deferral)
    qvt_sb = p_small.tile([128, BL * HK + HK], F32, tag="qvt")
    nc.scalar.dma_start(qvt_sb[:], qvt[:])
    qt_sb = qvt_sb[:, 0:BL * HK]
    vt_sb = qvt_sb[:, BL * HK:BL * HK + HK]
    mask_all = p_small.tile([1, BL * SP], U8, tag="mask")
    nc.scalar.dma_start(mask_all[:], masku[:])
    hsn_dma(0, eng=nc.sync)

    # PE clock warmup on garbage: dense N=128 matmuls until HAM hits K=8/8
    warm = pp_misc.tile([128, 128], F32, tag="ptr", name="warm")
    for _ in range(45):
        nc.tensor.matmul(warm[:], lhsT=wsrc[:], rhs=wsrc[:], start=True, stop=True)

    # ------------- pending-closure FIFO -------------
    pending = []

    def flush_one():
        if pending:
            pending.pop(0)()

    acc_t = {}

    def _act_chain_m(b, c, m, src, W):
        th = p_tanh.tile([128, W], F32, tag="tanh", name=f"th_{b}_{c}_{m}")
        nc.scalar.activation(
            th[:], src, AF.Tanh,
            bias=qt_sb[:, BL * m + b:BL * m + b + 1], scale=1.0,
        )
        if m == 0:
            acc = p_acc.tile([128, W], F32, tag="acc", name=f"acc_{b}_{c}")
            acc_t[(b, c)] = acc
            nc.vector.tensor_scalar_mul(acc[:], th[:], vt_sb[:, 0:1])
        elif m < HK - 1:
            acc = acc_t[(b, c)]
            nc.vector.scalar_tensor_tensor(
                acc[:], th[:], vt_sb[:, m:m + 1], acc[:], op0=OP.mult, op1=OP.add,
            )
        else:
            acc = acc_t.pop((b, c))
            acc_r = p_acc.tile([128, W], FP16, tag="accr", name=f"accr_{b}_{c}")
            acc_t[(b, c)] = acc_r
            nc.vector.scalar_tensor_tensor(
                acc_r[:], th[:], vt_sb[:, m:m + 1], acc[:], op0=OP.mult, op1=OP.add,
            )

    def p1_block00():
        """Block (0,0), DMA-paced: k-outer over 5 PSUM banks so the PE
        consumes hst chunks at arrival pace with no >1us idle windows
        (idle windows re-throttle HAM to 1.2GHz), then k-inner for the rest.
        All 8 pre tiles are DVE-evacuated to SBUF so the ppre banks recycle at
        DVE pace instead of waiting out the 8-tanh ACT burst at block end
        (which would stall (0,1)'s first m-groups ~5us)."""
        W, off = blocks[0], OFF[0]
        hstb = hstb_t[0]
        pre0 = p_small.tile([128, HK * W], F32, tag="pre0")
        pp = [
            pp_pre.tile([128, W], F32, tag="ppre", name=f"pp0_{m}")
            for m in range(MA)
        ]
        for k in range(HK):
            if k:
                # filler matmuls on garbage: the k-step is DMA-arrival-bound,
                # so these are free and keep HAM from re-throttling the PE
                # during the chunk gaps (early steps wait longest)
                for _ in range(12):
                    nc.tensor.matmul(warm[:], lhsT=wsrc[:], rhs=wsrc[:],
                                     start=True, stop=True)
            for m in range(MA):
                nc.tensor.matmul(
                    pp[m][:],
                    lhsT=w1_t[m][:, 128 * k:128 * (k + 1)],
                    rhs=hstb[k][:, off:off + W],
                    start=(k == 0), stop=(k == HK - 1),
                )
        for m in range(MA):
            nc.vector.tensor_copy(pre0[:, W * m:W * (m + 1)], pp[m][:])
        for m in range(MA, HK):
            ppre = pp_pre.tile([128, W], F32, tag="ppre", name=f"ppre_0_0_{m}")
            for k in range(HK):
                nc.tensor.matmul(
                    ppre[:],
                    lhsT=w1_t[m][:, 128 * k:128 * (k + 1)],
                    rhs=hstb[k][:, off:off + W],
                    start=(k == 0), stop=(k == HK - 1),
                )
            nc.vector.tensor_copy(pre0[:, W * m:W * (m + 1)], ppre[:])
        for m in range(HK):
            _act_chain_m(0, 0, m, pre0[:, W * m:W * (m + 1)], W)

    def p1_block(b, c, all_slots=False):
        W, off = blocks[c], OFF[c]
        if c == min(1, NSIG - 1) and b + 1 < BL:
            hst_dma(b + 1)
        slots = set(range(HK)) if all_slots else {1, 3, 5, 7}
        if NSIG == 2 and not all_slots:
            slots |= {2, 6}
        hstb = hstb_t[b]
        for m in range(HK):
            if m in slots:
                flush_one()
            ppre = pp_pre.tile([128, W], F32, tag="ppre", name=f"ppre_{b}_{c}_{m}")
            for k in range(HK):
                nc.tensor.matmul(
                    ppre[:],
                    lhsT=w1_t[m][:, 128 * k:128 * (k + 1)],
                    rhs=hstb[k][:, off:off + W],
                    start=(k == 0), stop=(k == HK - 1),
                )
            _act_chain_m(b, c, m, ppre[:], W)

    # ------------- per-block tail: tp -> tmid -> post -------------
    mx_t, zf_t, em_t, attZ_t, csum_t = {}, {}, {}, {}, {}

    def tail_tp(b, c):
        W, off = blocks[c], OFF[c]
        acc_r = acc_t.pop((b, c))
        pen = pp_misc.tile([1, W], F32, tag="ptr", name=f"pen_{b}_{c}")
        nc.tensor.matmul(pen[:], lhsT=ones_h[:], rhs=acc_r[:], start=True, stop=True)
        if c == 0:
            mx_t[b] = p_mx.tile([1, NSIG], F32, tag="mx", name=f"mx{b}")
            zf_t[b] = p_mx.tile([1, NSIG], F32, tag="zf", name=f"zf{b}")
        em = p_em.tile([1, W], F32, tag="em", name=f"em_{b}_{c}")
        nc.vector.scalar_tensor_tensor(
            em[:], mask_all[:, b * SP + off:b * SP + off + W], -1e10, pen[:],
            op0=OP.mult, op1=OP.add,
        )
        em_t[(b, c)] = em
        nc.vector.reduce_max(mx_t[b][:, c:c + 1], em[:], axis=AX.X)
        nmx = p_sc.tile([1, 1], F32, tag="nmx", name=f"nmx_{b}_{c}")
        nc.vector.tensor_scalar_mul(nmx[:], mx_t[b][:, c:c + 1], -1.0)
        nc.scalar.activation(
            em[:], em[:], AF.Exp, bias=nmx[:], scale=1.0,
            accum_out=zf_t[b][:, c:c + 1],
        )

    def tail_tmid(b, c):
        T = TPB[c]
        em = em_t.pop((b, c))
        if c == 0:
            attZ = p_attZ.tile([128, NSIG * NT], FP16, tag="attZ", name=f"attZ{b}")
            nc.vector.memset(attZ[:], 0.0)
            attZ_t[b] = attZ
        attZ = attZ_t[b]
        ptr = pp_misc.tile([128, T], F32, tag="ptr", name=f"ptr_{b}_{c}")
        for j in range(T):
            nc.tensor.transpose(ptr[:, j:j + 1], em[:, 128 * j:128 * (j + 1)], ident[:])
        for j in range(T):
            t = GBASE[c] + j
            nc.vector.tensor_copy(attZ[:, NSIG * t + c:NSIG * t + c + 1], ptr[:, j:j + 1])

    def tail_post(b, c):
        T = TPB[c]
        attZ = attZ_t[b]
        if c == 0:
            csum_t[b] = [
                pp_ctx.tile([NSIG, 512], F32, tag="pctx", name=f"csum_{b}_{n}")
                for n in range(2)
            ]
        csum = csum_t[b]
        tiles = hsn_t[b]
        for j in range(T):
            t = GBASE[c] + j
            for n in range(2):
                nc.tensor.matmul(
                    csum[n][:],
                    lhsT=attZ[:, NSIG * t:NSIG * (t + 1)],
                    rhs=tiles[t][:, 512 * n:512 * (n + 1)],
                    start=(c == 0 and j == 0), stop=(c == NSIG - 1 and j == T - 1),
                )

    # ------------- merge: merge_a (DVE chain + evac) / merge_b (matmuls) ----
    wrz_t, csr_t = {}, {}

    def merge_a(b):
        mx = mx_t.pop(b)
        zf = zf_t.pop(b)
        attZ_t.pop(b)
        csum = csum_t.pop(b)
        negM = p_sc.tile([1, 1], F32, tag="negM", name=f"negM{b}")
        nc.vector.reduce_max(negM[:], mx[:], axis=AX.X, negate=True)
        wt = p_mx.tile([1, NSIG], F32, tag="wt", name=f"wt{b}")
        nc.scalar.activation(wt[:], mx[:], AF.Exp, bias=negM[:], scale=1.0)
        zw = p_mx.tile([1, NSIG], F32, tag="zw", name=f"zw{b}")
        nc.vector.tensor_mul(zw[:], wt[:], zf[:])
        zs = p_sc.tile([1, 1], F32, tag="zs", name=f"zs{b}")
        nc.vector.reduce_sum(zs[:], zw[:], axis=AX.X)
        rz = p_sc.tile([1, 1], F32, tag="rz", name=f"rz{b}")
        nc.vector.reciprocal(rz[:], zs[:])
        wrz = p_mx.tile([1, NSIG], F32, tag="wrz", name=f"wrz{b}")
        nc.vector.tensor_scalar_mul(wrz[:], wt[:], rz[:])
        ptrw = pp_misc.tile([NSIG, 1], F32, tag="ptr", name=f"ptrw{b}")
        nc.tensor.transpose(ptrw[:], wrz[:], ident[:])
        wrz_h = p_mx.tile([NSIG, 1], FP16, tag="wrz_h", name=f"wrz_h{b}")
        nc.vector.tensor_copy(wrz_h[:], ptrw[:])
        wrz_t[b] = wrz_h
        csr = p_ctxs.tile([NSIG, H], FP16, tag="csr", name=f"csr{b}")
        for n in range(2):
            nc.vector.tensor_copy(csr[:, 512 * n:512 * (n + 1)], csum[n][:])
        csr_t[b] = csr

    def merge_b(b):
        wrz_h = wrz_t.pop(b)
        csr = csr_t.pop(b)
        cs = p_ctxs.tile([1, H], F32, tag="cs", name=f"cs{b}")
        # both matmuls back-to-back (mo1 borrows the pctx bank csum just
        # freed), then the two evacuations run on DVE and ACT in parallel
        mo0 = pp_misc.tile([1, 512], F32, tag="ptr", name=f"mo_{b}_0")
        mo1 = pp_ctx.tile([1, 512], F32, tag="pctx", name=f"mo_{b}_1")
        nc.tensor.matmul(mo0[:], lhsT=wrz_h[:], rhs=csr[:, 0:512],
                         start=True, stop=True)
        nc.tensor.matmul(mo1[:], lhsT=wrz_h[:], rhs=csr[:, 512:1024],
                         start=True, stop=True)
        nc.vector.tensor_copy(cs[:, 0:512], mo0[:])
        nc.scalar.activation(cs[:, 512:1024], mo1[:], AF.Copy)
        nc.sync.dma_start(ctx_out[b:b + 1, 0:512], cs[:, 0:512])
        nc.sync.dma_start(ctx_out[b:b + 1, 512:1024], cs[:, 512:1024])

    def append_tail(b, c):
        pending.append(lambda: tail_tp(b, c))
        pending.append(lambda: tail_tmid(b, c))
        pending.append(lambda: tail_post(b, c))
        if c == 0 and b + 1 < BL:
            # hsn prefetch goes through the FIFO so it is traced AFTER the
            # previous batch's context matmuls released the hsn slots.
            pending.append(lambda: hsn_dma(b + 1))

    # ------------- schedule -------------
    p1_block00()
    if NSIG == 1:
        hst_dma(1)
    append_tail(0, 0)
    for c in range(1, NSIG):
        p1_block(0, c)
        append_tail(0, c)
    pending.append(lambda: merge_a(0))
    pending.append(lambda: merge_b(0))
    for b in range(1, BL):
        for c in range(NSIG):
            p1_block(b, c, all_slots=(b == BL - 1))
            append_tail(b, c)
        pending.append(lambda bb=b: merge_a(bb))
        pending.append(lambda bb=b: merge_b(bb))
    while pending:
        flush_one()


def build_program(blocks):
    blocks = tuple(blocks)
    key = ("nc", blocks)
    if key in _CACHE:
        return _CACHE[key]
    SP = sum(blocks)
    nc = bacc.Bacc("TRN2", target_bir_lowering=False, debug=False, enable_asserts=False)
    NT = SP // 128
    aps = {
        "hst": nc.dram_tensor("hst", (BL, 128, HK * SP), FP16, kind="ExternalInput").ap(),
        "hsn": nc.dram_tensor("hsn", (BL, 128, NT * H), FP16, kind="ExternalInput").ap(),
        "w1r": nc.dram_tensor("w1r", (128, HK * H), FP16, kind="ExternalInput").ap(),
        "qvt": nc.dram_tensor("qvt", (128, BL * HK + HK), F32, kind="ExternalInput").ap(),
        "masku": nc.dram_tensor("masku", (1, BL * SP), U8, kind="ExternalInput").ap(),
        "ctx": nc.dram_tensor("ctx", (BL, H), F32, kind="ExternalOutput").ap(),
    }
    with tile.TileContext(nc) as tc:
        with ExitStack() as stack:
            aps["ctx_stack"] = stack
            _emit(tc, aps, blocks)
    nc.compile()
    _CACHE[key] = nc
    return nc


def prep_in_maps(inputs, blocks):
    SP = sum(blocks)
    hidden = np.ascontiguousarray(np.asarray(inputs["hidden"], dtype=np.float32))
    hs = np.asarray(inputs["hidden_sequence"], dtype=np.float32)
    masks = np.asarray(inputs["input_masks"]).astype(bool)
    w1t = np.asarray(inputs["W1"], dtype=np.float32).T  # (hin, hout)
    # m-major relayout: w1r[:, 1024*m + 128*k : +128] = W1T[128k:128(k+1), 128m:128(m+1)]
    w1r = np.ascontiguousarray(
        w1t.reshape(HK, 128, HK, 128).transpose(1, 2, 0, 3).reshape(128, HK * H)
        .astype(np.float16)
    )
    b1 = np.asarray(inputs["b1"], dtype=np.float32)
    b2 = np.asarray(inputs["b2"], dtype=np.float32)
    v = np.asarray(inputs["v"], dtype=np.float32)
    # q = W2 @ hidden + b1 + b2 on host (67 MFLOP, 0.004% of the kernel's
    # FLOPs) -- removes the 2MB W2 DMA + 4.3us of PE work from the device
    w2 = np.asarray(inputs["W2"], dtype=np.float32)
    q_all = hidden[0] @ w2.T + b1 + b2          # (B, H)
    vt = v.reshape(HK, 128).T                   # (128, HK)
    in_maps = []
    for ci in range(NCORES):
        g = slice(BL * ci, BL * (ci + 1))
        # qt[p, BL*m + b] = q_all[g][b, 128m + p]
        qt = q_all[g].reshape(BL, HK, 128).transpose(2, 1, 0).reshape(128, HK * BL)
        qvt = np.ascontiguousarray(
            np.concatenate([qt, vt], axis=1).astype(np.float32))
        NT = SP // 128
        hst_p = np.zeros((BL, H, SP), dtype=np.float16)
        hsn_p = np.zeros((BL, SP, H), dtype=np.float16)
        maskp = np.ones((BL, SP), dtype=np.uint8)  # 1 = padded (masked out)
        for b in range(BL):
            gb = BL * ci + b
            idx = np.nonzero(masks[:, gb])[0]
            n = idx.shape[0]
            sel = hs[idx, gb, :].astype(np.float16)  # (n, H)
            hst_p[b, :, :n] = sel.T
            hsn_p[b, :n, :] = sel
            maskp[b, :n] = 0
        # fat-descriptor relayouts: (BL,128,HK*SP) and (BL,128,NT*H)
        hst_r = np.ascontiguousarray(
            hst_p.reshape(BL, HK, 128, SP).transpose(0, 2, 1, 3)
            .reshape(BL, 128, HK * SP))
        hsn_r = np.ascontiguousarray(
            hsn_p.reshape(BL, NT, 128, H).transpose(0, 2, 1, 3)
            .reshape(BL, 128, NT * H))
        in_maps.append({
            "hst": hst_r,
            "hsn": hsn_r,
            "w1r": w1r,
            "qvt": qvt,
            "masku": maskp.reshape(1, BL * SP),
        })
    return in_maps


def kernel(**inputs):
    blocks = pick_blocks(inputs["input_masks"])
    nc = build_program(blocks)
    in_maps = prep_in_maps(inputs, blocks)
    res = bass_utils.run_bass_kernel_spmd(nc, in_maps, list(range(NCORES)))
    out = np.concatenate([res.results[i]["ctx"] for i in range(NCORES)], axis=0)
    return out[None].astype(np.float32)


if __name__ == "__main__":
    build_program((512, 512, 128))
    print("program built OK")


# revision 58
# speedup vs baseline: 1.1353x; 1.0334x over previous
"""Bahdanau additive attention on TRN2, data-parallel over batch on 8 NeuronCores.

Reference computation (per batch b):
    pre[s, :]  = W1 @ hs[s, b, :] + b1 + W2 @ hidden[b, :] + b2      # (S, H)
    energy[s]  = v . tanh(pre[s, :])                                  # (S,)
    energy     = where(mask[s, b], energy, -1e10)
    attn       = softmax(energy over s)
    ctx[b, :]  = sum_s attn[s] * hs[s, b, :]                          # (H,)

Sparsity: masked positions get attn == 0 exactly, so the host packs only the
unmasked rows of hs per batch (~50% of S) into a fixed SP-wide layout, padded
with zero columns that the device masks out of the softmax.

Per-core layout (batch-sharded, BL=4 batches per core):
  - Sigma blocks are variable width [512, 512(, 128...)] chosen from the
    actual max unmasked count: wide matmuls amortize the ~35ns fixed
    per-matmul overhead (measured: N=384 -> 195ns, N=512 -> 240ns, and the
    overhead is NOT LDWEIGHTS -- repeating the same stationary doesn't help).
  - Every DMA'd operand is its own tile (w1 per m-chunk, hst per k-chunk,
    hsn per s-tile) so Tile's dependency tracking is per-chunk and the first
    matmul only gates on w1[m0] + hst[k0] instead of the whole 4.6MB.
  - PE warmup runs ~45 N=128 matmuls on memset garbage so HAM reaches
    K=8/8 before the first real matmul (no DMA dependency at all).
  - The block tail is split into three closures flushed at separate slots in
    later blocks' matmul streams: tp (pen matmul + mask + max + exp),
    tmid (PE transposes + attZ copies), post (context matmuls). This gives
    the ACT/DVE chain 2+ m-groups of latency headroom so the in-order PE
    queue never stalls on it (the baseline lost ~1.9us per block here).
    The merge is likewise split into merge_a (DVE chain + csum->SBUF
    evacuation) and merge_b (the weighted-merge matmuls + output DMA).
  - pen / merge matmuls take fp16 moving operands (f32r streams at 1/4 rate
    at full clock for narrow tiles; fp16 error on the energy is ~1e-3,
    far inside the 2e-2 gate).
"""

import sys
from contextlib import ExitStack

import numpy as np

if "/opt/trn_rl_repo" not in sys.path:
    sys.path.append("/opt/trn_rl_repo")

import concourse.bass as bass
import concourse.bacc as bacc
import concourse.mybir as mybir
import concourse.tile as tile
from concourse import bass_utils

S, B, H = 2048, 32, 1024
NCORES = 8
BL = B // NCORES  # local batches per core
HK = H // 128     # 128-partition chunks of H

F32 = mybir.dt.float32
F32R = mybir.dt.float32r
U8 = mybir.dt.uint8
FP16 = mybir.dt.float16
AF = mybir.ActivationFunctionType
AX = mybir.AxisListType
OP = mybir.AluOpType

_CACHE = {}


def pick_blocks(masks):
    """Sigma-block widths [512]*q + [128*r]: minimal 128-multiple >= maxn."""
    maxn = int(np.asarray(masks).sum(axis=0).max())
    maxn = max(maxn, 128)
    q, rem = divmod(maxn, 512)
    blocks = [512] * q
    if rem:
        blocks.append(128 * ((rem + 127) // 128))
    return tuple(blocks)


def _emit(tc, aps, blocks):
    nc = tc.nc
    ctx = aps["ctx_stack"]
    NSIG = len(blocks)
    SP = sum(blocks)
    OFF = [sum(blocks[:c]) for c in range(NSIG)]
    TPB = [w // 128 for w in blocks]
    GBASE = [OFF[c] // 128 for c in range(NSIG)]
    NT = SP // 128
    hst, hsn, w1r, qvt, masku, ctx_out = (
        aps["hst"], aps["hsn"], aps["w1r"],
        aps["qvt"], aps["masku"], aps["ctx"],
    )

    def pool(name, bufs, space="SBUF"):
        return ctx.enter_context(tc.tile_pool(name=name, bufs=bufs, space=space))

    big = SP > 1152  # rare dense-mask case: trade overlap for SBUF headroom
    p_w1 = pool("w1", 1)
    p_hst = pool("hst", 2)     # half-batch tiles, 2 batches in flight
    p_hsn = pool("hsn", 1 if SP > 1280 else 2)
    p_small = pool("small", 1)
    p_tanh = pool("tanh", 3)
    p_acc = pool("acc", 6)
    p_em = pool("em", 3)
    p_ctxs = pool("ctxs", 2)
    p_attZ = pool("attZ", 2)
    p_sc = pool("sc", 4)
    p_mx = pool("mx", 8)

    pp_pre = pool("ppre", 5, space="PSUM")
    pp_misc = pool("pmisc", 1, space="PSUM")
    pp_ctx = pool("pctx", 2, space="PSUM")

    # ---------------- setup DMAs ----------------
    # Host relays every big operand so one DMA moves 9-18KB per partition
    # (fat descriptors ~300GB/s; the naive per-row 2KB descriptors cap a
    # queue row at ~60-100GB/s).  hst is split in two half-batch tiles so
    # block (0,0)'s first k-chunks gate on only half the bytes.
    MA = 4  # phase-A m-count for block (0,0): phase A gates on w1[m0..m3]
            # (1MB on sync) while w1[m4..m7] stream on scalar for phase B
    w1_t = [
        p_w1.tile([128, H], FP16, tag=f"w1_{m}", name=f"w1_{m}", bufs=1)
        for m in range(HK)
    ]
    hstb_t = {}

    HA = HK // 2

    def hst_dma(b, eng=None):
        """Fat half-batch transfers (steady-state batches), split over two
        queue rows (each row gets ~1/3 of aggregate DMA bandwidth)."""
        ta = p_hst.tile([128, HA * SP], FP16, tag="hstr_a", name=f"hst_{b}_a")
        tb = p_hst.tile([128, (HK - HA) * SP], FP16, tag="hstr_b", name=f"hst_{b}_b")
        (eng or nc.gpsimd).dma_start(ta[:], hst[b, :, :HA * SP])
        (eng or nc.scalar).dma_start(tb[:], hst[b, :, HA * SP:])
        hstb_t[b] = [
            (ta[:, SP * k:SP * (k + 1)] if k < HA
             else tb[:, SP * (k - HA):SP * (k - HA + 1)])
            for k in range(HK)
        ]

    hsn_t = {}

    def hsn_dma(b, eng=None):
        h = p_hsn.tile([128, NT * H], FP16, tag="hsnr", name=f"hsn_{b}")
        (eng or nc.sync).dma_start(h[:], hsn[b, :, :])
        hsn_t[b] = [h[:, H * t:H * (t + 1)] for t in range(NT)]

    # warmup source: memset garbage, no DMA dependency.  On DVE -- the
    # gpsimd engine queue must start with its dma_starts (SWDGE) or the
    # first gpsimd-row DMA lands several us late.
    wsrc = p_small.tile([128, 128], FP16, tag="wsrc")
    nc.vector.memset(wsrc[:], 0.5)
    ident = p_small.tile([1, 1], F32, tag="ident")
    nc.vector.memset(ident[:], 1.0)
    ones_h = p_small.tile([128, 1], FP16, tag="ones_h")
    nc.vector.memset(ones_h[:], 1.0)

    # Head-priority DMAs.  Block (0,0) phase A consumes w1[m] x hst0[k]
    # progressively, so both are per-chunk tiles emitted in consumption
    # order, interleaved across the three queue rows (each row gets ~1/3 of
    # the ~300GB/s aggregate while all are busy).
    def w1_dma(m):
        nc.sync.dma_start(w1_t[m][:], w1r[:, H * m:H * (m + 1)])

    if big:
        w1_dma(0)
        hst_dma(0)  # fat halves on gpsimd+scalar; phase A paces off arrival
        for m in range(1, HK):
            w1_dma(m)
    else:
        tiles0 = [
            p_hst.tile([128, SP], FP16, tag=f"h0k{k}", name=f"hst_0_{k}", bufs=1)
            for k in range(HK)
        ]
        hstb_t[0] = [t[:] for t in tiles0]

        def hst0_dma(k, eng):
            eng.dma_start(tiles0[k][:], hst[0, :, SP * k:SP * (k + 1)])

        # sync: w1 m0-m3 (phase A) | scalar: chunks k0,k1,k2,k5 + smalls +
        # w1 m4-m7 (phase B, just-in-time) | gpsimd: chunks k3,k4,k6,k7
        w1_dma(0)
        hst0_dma(0, nc.scalar)
        hst0_dma(3, nc.gpsimd)
        w1_dma(1)
        hst0_dma(1, nc.scalar)
        hst0_dma(4, nc.gpsimd)
        w1_dma(2)
        hst0_dma(2, nc.scalar)
        hst0_dma(6, nc.gpsimd)
        w1_dma(3)
        hst0_dma(5, nc.scalar)
        hst0_dma(7, nc.gpsimd)

    # qvt = [qt | vt] precomputed on host (q = W2 @ hidden + b1 + b2 is only
    # 0.004% of total FLOPs; computing it host-side removes the 2MB W2 DMA
    # from the critical window and the whole batch-0 activation deferral)
    qvt_sb = p_small.tile([128, BL * HK + HK], F32, tag="qvt")
    nc.scalar.dma_start(qvt_sb[:], qvt[:])
    qt_sb = qvt_sb[:, 0:BL * HK]
    vt_sb = qvt_sb[:, BL * HK:BL * HK + HK]
    mask_all = p_small.tile([1, BL * SP], U8, tag="mask")
    nc.scalar.dma_start(mask_all[:], masku[:])
    if not big:
        for m in range(MA, HK):
            nc.scalar.dma_start(w1_t[m][:], w1r[:, H * m:H * (m + 1)])
    hsn_dma(0, eng=nc.sync)

    # PE clock warmup on garbage: dense N=128 matmuls until HAM hits K=8/8
    warm = pp_misc.tile([128, 128], F32, tag="ptr", name="warm")
    for _ in range(45):
        nc.tensor.matmul(warm[:], lhsT=wsrc[:], rhs=wsrc[:], start=True, stop=True)

    # ------------- pending-closure FIFO -------------
    pending = []

    def flush_one():
        if pending:
            pending.pop(0)()

    acc_t = {}

    def _act_chain_m(b, c, m, src, W):
        th = p_tanh.tile([128, W], F32, tag="tanh", name=f"th_{b}_{c}_{m}")
        nc.scalar.activation(
            th[:], src, AF.Tanh,
            bias=qt_sb[:, BL * m + b:BL * m + b + 1], scale=1.0,
        )
        if m == 0:
            acc = p_acc.tile([128, W], F32, tag="acc", name=f"acc_{b}_{c}")
            acc_t[(b, c)] = acc
            nc.vector.tensor_scalar_mul(acc[:], th[:], vt_sb[:, 0:1])
        elif m < HK - 1:
            acc = acc_t[(b, c)]
            nc.vector.scalar_tensor_tensor(
                acc[:], th[:], vt_sb[:, m:m + 1], acc[:], op0=OP.mult, op1=OP.add,
            )
        else:
            acc = acc_t.pop((b, c))
            acc_r = p_acc.tile([128, W], FP16, tag="accr", name=f"accr_{b}_{c}")
            acc_t[(b, c)] = acc_r
            nc.vector.scalar_tensor_tensor(
                acc_r[:], th[:], vt_sb[:, m:m + 1], acc[:], op0=OP.mult, op1=OP.add,
            )

    def p1_block00():
        """Block (0,0), DMA-paced: k-outer over 5 PSUM banks so the PE
        consumes hst chunks at arrival pace with no >1us idle windows
        (idle windows re-throttle HAM to 1.2GHz), then k-inner for the rest.
        All 8 pre tiles are DVE-evacuated to SBUF so the ppre banks recycle at
        DVE pace instead of waiting out the 8-tanh ACT burst at block end
        (which would stall (0,1)'s first m-groups ~5us)."""
        W, off = blocks[0], OFF[0]
        hstb = hstb_t[0]
        pre0 = p_small.tile([128, HK * W], F32, tag="pre0")
        pp = [
            pp_pre.tile([128, W], F32, tag="ppre", name=f"pp0_{m}")
            for m in range(MA)
        ]
        # consume chunks in DMA-arrival order (scalar row: k0,k1,k2,k5
        # interleaving with gpsimd row: k3,k4,k6,k7); the PSUM accumulation
        # order over k is irrelevant
        korder = [0, 3, 1, 4, 2, 6, 5, 7] if not big else list(range(HK))
        for ki, k in enumerate(korder):
            if ki:
                # filler matmuls on garbage: the k-step is DMA-arrival-bound,
                # so these are free and keep HAM from re-throttling the PE
                # during the chunk gaps (early steps wait longest)
                for _ in range(12):
                    nc.tensor.matmul(warm[:], lhsT=wsrc[:], rhs=wsrc[:],
                                     start=True, stop=True)
            for m in range(MA):
                nc.tensor.matmul(
                    pp[m][:],
                    lhsT=w1_t[m][:, 128 * k:128 * (k + 1)],
                    rhs=hstb[k][:, off:off + W],
                    start=(ki == 0), stop=(ki == HK - 1),
                )
        for m in range(MA):
            nc.vector.tensor_copy(pre0[:, W * m:W * (m + 1)], pp[m][:])
        for m in range(MA, HK):
            ppre = pp_pre.tile([128, W], F32, tag="ppre", name=f"ppre_0_0_{m}")
            for k in range(HK):
                nc.tensor.matmul(
                    ppre[:],
                    lhsT=w1_t[m][:, 128 * k:128 * (k + 1)],
                    rhs=hstb[k][:, off:off + W],
                    start=(k == 0), stop=(k == HK - 1),
                )
            nc.vector.tensor_copy(pre0[:, W * m:W * (m + 1)], ppre[:])
        for m in range(HK):
            _act_chain_m(0, 0, m, pre0[:, W * m:W * (m + 1)], W)

    def p1_block(b, c, all_slots=False):
        W, off = blocks[c], OFF[c]
        if c == min(1, NSIG - 1) and b + 1 < BL:
            hst_dma(b + 1)
        slots = set(range(HK)) if all_slots else {1, 3, 5, 7}
        if NSIG == 2 and not all_slots:
            slots |= {2, 6}
        hstb = hstb_t[b]
        for m in range(HK):
            if m in slots:
                flush_one()
            ppre = pp_pre.tile([128, W], F32, tag="ppre", name=f"ppre_{b}_{c}_{m}")
            for k in range(HK):
                nc.tensor.matmul(
                    ppre[:],
                    lhsT=w1_t[m][:, 128 * k:128 * (k + 1)],
                    rhs=hstb[k][:, off:off + W],
                    start=(k == 0), stop=(k == HK - 1),
                )
            _act_chain_m(b, c, m, ppre[:], W)

    # ------------- per-block tail: tp -> tmid -> post -------------
    mx_t, zf_t, em_t, attZ_t, csum_t = {}, {}, {}, {}, {}

    def tail_tp(b, c):
        W, off = blocks[c], OFF[c]
        acc_r = acc_t.pop((b, c))
        pen = pp_misc.tile([1, W], F32, tag="ptr", name=f"pen_{b}_{c}")
        nc.tensor.matmul(pen[:], lhsT=ones_h[:], rhs=acc_r[:], start=True, stop=True)
        if c == 0:
            mx_t[b] = p_mx.tile([1, NSIG], F32, tag="mx", name=f"mx{b}")
            zf_t[b] = p_mx.tile([1, NSIG], F32, tag="zf", name=f"zf{b}")
        em = p_em.tile([1, W], F32, tag="em", name=f"em_{b}_{c}")
        nc.vector.scalar_tensor_tensor(
            em[:], mask_all[:, b * SP + off:b * SP + off + W], -1e10, pen[:],
            op0=OP.mult, op1=OP.add,
        )
        em_t[(b, c)] = em
        nc.vector.reduce_max(mx_t[b][:, c:c + 1], em[:], axis=AX.X)
        nmx = p_sc.tile([1, 1], F32, tag="nmx", name=f"nmx_{b}_{c}")
        nc.vector.tensor_scalar_mul(nmx[:], mx_t[b][:, c:c + 1], -1.0)
        nc.scalar.activation(
            em[:], em[:], AF.Exp, bias=nmx[:], scale=1.0,
            accum_out=zf_t[b][:, c:c + 1],
        )

    def tail_tmid(b, c):
        T = TPB[c]
        em = em_t.pop((b, c))
        if c == 0:
            attZ = p_attZ.tile([128, NSIG * NT], FP16, tag="attZ", name=f"attZ{b}")
            nc.vector.memset(attZ[:], 0.0)
            attZ_t[b] = attZ
        attZ = attZ_t[b]
        ptr = pp_misc.tile([128, T], F32, tag="ptr", name=f"ptr_{b}_{c}")
        for j in range(T):
            nc.tensor.transpose(ptr[:, j:j + 1], em[:, 128 * j:128 * (j + 1)], ident[:])
        for j in range(T):
            t = GBASE[c] + j
            nc.vector.tensor_copy(attZ[:, NSIG * t + c:NSIG * t + c + 1], ptr[:, j:j + 1])

    def tail_post(b, c):
        T = TPB[c]
        attZ = attZ_t[b]
        if c == 0:
            csum_t[b] = [
                pp_ctx.tile([NSIG, 512], F32, tag="pctx", name=f"csum_{b}_{n}")
                for n in range(2)
            ]
        csum = csum_t[b]
        tiles = hsn_t[b]
        for j in range(T):
            t = GBASE[c] + j
            for n in range(2):
                nc.tensor.matmul(
                    csum[n][:],
                    lhsT=attZ[:, NSIG * t:NSIG * (t + 1)],
                    rhs=tiles[t][:, 512 * n:512 * (n + 1)],
                    start=(c == 0 and j == 0), stop=(c == NSIG - 1 and j == T - 1),
                )

    # ------------- merge: merge_a (DVE chain + evac) / merge_b (matmuls) ----
    wrz_t, csr_t = {}, {}

    def merge_a(b):
        mx = mx_t.pop(b)
        zf = zf_t.pop(b)
        attZ_t.pop(b)
        csum = csum_t.pop(b)
        negM = p_sc.tile([1, 1], F32, tag="negM", name=f"negM{b}")
        nc.vector.reduce_max(negM[:], mx[:], axis=AX.X, negate=True)
        wt = p_mx.tile([1, NSIG], F32, tag="wt", name=f"wt{b}")
        nc.scalar.activation(wt[:], mx[:], AF.Exp, bias=negM[:], scale=1.0)
        zw = p_mx.tile([1, NSIG], F32, tag="zw", name=f"zw{b}")
        nc.vector.tensor_mul(zw[:], wt[:], zf[:])
        zs = p_sc.tile([1, 1], F32, tag="zs", name=f"zs{b}")
        nc.vector.reduce_sum(zs[:], zw[:], axis=AX.X)
        rz = p_sc.tile([1, 1], F32, tag="rz", name=f"rz{b}")
        nc.vector.reciprocal(rz[:], zs[:])
        wrz = p_mx.tile([1, NSIG], F32, tag="wrz", name=f"wrz{b}")
        nc.vector.tensor_scalar_mul(wrz[:], wt[:], rz[:])
        ptrw = pp_misc.tile([NSIG, 1], F32, tag="ptr", name=f"ptrw{b}")
        nc.tensor.transpose(ptrw[:], wrz[:], ident[:])
        wrz_h = p_mx.tile([NSIG, 1], FP16, tag="wrz_h", name=f"wrz_h{b}")
        nc.vector.tensor_copy(wrz_h[:], ptrw[:])
        wrz_t[b] = wrz_h
        csr = p_ctxs.tile([NSIG, H], FP16, tag="csr", name=f"csr{b}")
        for n in range(2):
            nc.vector.tensor_copy(csr[:, 512 * n:512 * (n + 1)], csum[n][:])
        csr_t[b] = csr

    def merge_b(b):
        wrz_h = wrz_t.pop(b)
        csr = csr_t.pop(b)
        cs = p_ctxs.tile([1, H], F32, tag="cs", name=f"cs{b}")
        # both matmuls back-to-back (mo1 borrows the pctx bank csum just
        # freed), then the two evacuations run on DVE and ACT in parallel
        mo0 = pp_misc.tile([1, 512], F32, tag="ptr", name=f"mo_{b}_0")
        mo1 = pp_ctx.tile([1, 512], F32, tag="pctx", name=f"mo_{b}_1")
        nc.tensor.matmul(mo0[:], lhsT=wrz_h[:], rhs=csr[:, 0:512],
                         start=True, stop=True)
        nc.tensor.matmul(mo1[:], lhsT=wrz_h[:], rhs=csr[:, 512:1024],
                         start=True, stop=True)
        nc.vector.tensor_copy(cs[:, 0:512], mo0[:])
        nc.scalar.activation(cs[:, 512:1024], mo1[:], AF.Copy)
        nc.sync.dma_start(ctx_out[b:b + 1, 0:512], cs[:, 0:512])
        nc.sync.dma_start(ctx_out[b:b + 1, 512:1024], cs[:, 512:1024])

    def append_tail(b, c):
        pending.append(lambda: tail_tp(b, c))
        pending.append(lambda: tail_tmid(b, c))
        pending.append(lambda: tail_post(b, c))
        if c == 0 and b + 1 < BL:
            # hsn prefetch goes through the FIFO so it is traced AFTER the
            # previous batch's context matmuls released the hsn slots.
            pending.append(lambda: hsn_dma(b + 1))

    # ------------- schedule -------------
    p1_block00()
    if NSIG == 1:
        hst_dma(1)
    append_tail(0, 0)
    for c in range(1, NSIG):
        p1_block(0, c)
        append_tail(0, c)
    pending.append(lambda: merge_a(0))
    pending.append(lambda: merge_b(0))
    for b in range(1, BL):
        for c in range(NSIG):
            p1_block(b, c, all_slots=(b == BL - 1))
            append_tail(b, c)
        pending.append(lambda bb=b: merge_a(bb))
        pending.append(lambda bb=b: merge_b(bb))
    while pending:
        flush_one()


def build_program(blocks):
    blocks = tuple(blocks)
    key = ("nc", blocks)
    if key in _CACHE:
        return _CACHE[key]
    SP = sum(blocks)
    nc = bacc.Bacc("TRN2", target_bir_lowering=False, debug=False, enable_asserts=False)
    NT = SP // 128
    aps = {
        "hst": nc.dram_tensor("hst", (BL, 128, HK * SP), FP16, kind="ExternalInput").ap(),
        "hsn": nc.dram_tensor("hsn", (BL, 128, NT * H), FP16, kind="ExternalInput").ap(),
        "w1r": nc.dram_tensor("w1r", (128, HK * H), FP16, kind="ExternalInput").ap(),
        "qvt": nc.dram_tensor("qvt", (128, BL * HK + HK), F32, kind="ExternalInput").ap(),
        "masku": nc.dram_tensor("masku", (1, BL * SP), U8, kind="ExternalInput").ap(),
        "ctx": nc.dram_tensor("ctx", (BL, H), F32, kind="ExternalOutput").ap(),
    }
    with tile.TileContext(nc) as tc:
        with ExitStack() as stack:
            aps["ctx_stack"] = stack
            _emit(tc, aps, blocks)
    nc.compile()
    _CACHE[key] = nc
    return nc


def prep_in_maps(inputs, blocks):
    SP = sum(blocks)
    hidden = np.ascontiguousarray(np.asarray(inputs["hidden"], dtype=np.float32))
    hs = np.asarray(inputs["hidden_sequence"], dtype=np.float32)
    masks = np.asarray(inputs["input_masks"]).astype(bool)
    w1t = np.asarray(inputs["W1"], dtype=np.float32).T  # (hin, hout)
    # m-major relayout: w1r[:, 1024*m + 128*k : +128] = W1T[128k:128(k+1), 128m:128(m+1)]
    w1r = np.ascontiguousarray(
        w1t.reshape(HK, 128, HK, 128).transpose(1, 2, 0, 3).reshape(128, HK * H)
        .astype(np.float16)
    )
    b1 = np.asarray(inputs["b1"], dtype=np.float32)
    b2 = np.asarray(inputs["b2"], dtype=np.float32)
    v = np.asarray(inputs["v"], dtype=np.float32)
    # q = W2 @ hidden + b1 + b2 on host (67 MFLOP, 0.004% of the kernel's
    # FLOPs) -- removes the 2MB W2 DMA + 4.3us of PE work from the device
    w2 = np.asarray(inputs["W2"], dtype=np.float32)
    q_all = hidden[0] @ w2.T + b1 + b2          # (B, H)
    vt = v.reshape(HK, 128).T                   # (128, HK)
    in_maps = []
    for ci in range(NCORES):
        g = slice(BL * ci, BL * (ci + 1))
        # qt[p, BL*m + b] = q_all[g][b, 128m + p]
        qt = q_all[g].reshape(BL, HK, 128).transpose(2, 1, 0).reshape(128, HK * BL)
        qvt = np.ascontiguousarray(
            np.concatenate([qt, vt], axis=1).astype(np.float32))
        NT = SP // 128
        hst_p = np.zeros((BL, H, SP), dtype=np.float16)
        hsn_p = np.zeros((BL, SP, H), dtype=np.float16)
        maskp = np.ones((BL, SP), dtype=np.uint8)  # 1 = padded (masked out)
        for b in range(BL):
            gb = BL * ci + b
            idx = np.nonzero(masks[:, gb])[0]
            n = idx.shape[0]
            sel = hs[idx, gb, :].astype(np.float16)  # (n, H)
            hst_p[b, :, :n] = sel.T
            hsn_p[b, :n, :] = sel
            maskp[b, :n] = 0
        # fat-descriptor relayouts: (BL,128,HK*SP) and (BL,128,NT*H)
        hst_r = np.ascontiguousarray(
            hst_p.reshape(BL, HK, 128, SP).transpose(0, 2, 1, 3)
            .reshape(BL, 128, HK * SP))
        hsn_r = np.ascontiguousarray(
            hsn_p.reshape(BL, NT, 128, H).transpose(0, 2, 1, 3)
            .reshape(BL, 128, NT * H))
        in_maps.append({
            "hst": hst_r,
            "hsn": hsn_r,
            "w1r": w1r,
            "qvt": qvt,
            "masku": maskp.reshape(1, BL * SP),
        })
    return in_maps


def kernel(**inputs):
    blocks = pick_blocks(inputs["input_masks"])
    nc = build_program(blocks)
    in_maps = prep_in_maps(inputs, blocks)
    res = bass_utils.run_bass_kernel_spmd(nc, in_maps, list(range(NCORES)))
    out = np.concatenate([res.results[i]["ctx"] for i in range(NCORES)], axis=0)
    return out[None].astype(np.float32)


if __name__ == "__main__":
    build_program((512, 512, 128))
    print("program built OK")


# revision 68
# speedup vs baseline: 1.1371x; 1.0016x over previous
"""Bahdanau additive attention on TRN2, data-parallel over batch on 8 NeuronCores.

Reference computation (per batch b):
    pre[s, :]  = W1 @ hs[s, b, :] + b1 + W2 @ hidden[b, :] + b2      # (S, H)
    energy[s]  = v . tanh(pre[s, :])                                  # (S,)
    energy     = where(mask[s, b], energy, -1e10)
    attn       = softmax(energy over s)
    ctx[b, :]  = sum_s attn[s] * hs[s, b, :]                          # (H,)

Sparsity: masked positions get attn == 0 exactly, so the host packs only the
unmasked rows of hs per batch (~50% of S) into a fixed SP-wide layout, padded
with zero columns that the device masks out of the softmax.

Per-core layout (batch-sharded, BL=4 batches per core):
  - Sigma blocks are variable width [512, 512(, 128...)] chosen from the
    actual max unmasked count: wide matmuls amortize the ~35ns fixed
    per-matmul overhead (measured: N=384 -> 195ns, N=512 -> 240ns, and the
    overhead is NOT LDWEIGHTS -- repeating the same stationary doesn't help).
  - Every DMA'd operand is its own tile (w1 per m-chunk, hst per k-chunk,
    hsn per s-tile) so Tile's dependency tracking is per-chunk and the first
    matmul only gates on w1[m0] + hst[k0] instead of the whole 4.6MB.
  - PE warmup runs ~45 N=128 matmuls on memset garbage so HAM reaches
    K=8/8 before the first real matmul (no DMA dependency at all).
  - The block tail is split into three closures flushed at separate slots in
    later blocks' matmul streams: tp (pen matmul + mask + max + exp),
    tmid (PE transposes + attZ copies), post (context matmuls). This gives
    the ACT/DVE chain 2+ m-groups of latency headroom so the in-order PE
    queue never stalls on it (the baseline lost ~1.9us per block here).
    The merge is likewise split into merge_a (DVE chain + csum->SBUF
    evacuation) and merge_b (the weighted-merge matmuls + output DMA).
  - pen / merge matmuls take fp16 moving operands (f32r streams at 1/4 rate
    at full clock for narrow tiles; fp16 error on the energy is ~1e-3,
    far inside the 2e-2 gate).
"""

import sys
from contextlib import ExitStack

import numpy as np

if "/opt/trn_rl_repo" not in sys.path:
    sys.path.append("/opt/trn_rl_repo")

import concourse.bass as bass
import concourse.bacc as bacc
import concourse.mybir as mybir
import concourse.tile as tile
from concourse import bass_utils

S, B, H = 2048, 32, 1024
NCORES = 8
BL = B // NCORES  # local batches per core
HK = H // 128     # 128-partition chunks of H

F32 = mybir.dt.float32
F32R = mybir.dt.float32r
U8 = mybir.dt.uint8
FP16 = mybir.dt.float16
AF = mybir.ActivationFunctionType
AX = mybir.AxisListType
OP = mybir.AluOpType

_CACHE = {}


def pick_blocks(masks):
    """Sigma-block widths [512]*q + [128*r]: minimal 128-multiple >= maxn."""
    maxn = int(np.asarray(masks).sum(axis=0).max())
    maxn = max(maxn, 128)
    q, rem = divmod(maxn, 512)
    blocks = [512] * q
    if rem:
        blocks.append(128 * ((rem + 127) // 128))
    return tuple(blocks)


def _emit(tc, aps, blocks, use_mask):
    nc = tc.nc
    ctx = aps["ctx_stack"]
    NSIG = len(blocks)
    SP = sum(blocks)
    OFF = [sum(blocks[:c]) for c in range(NSIG)]
    TPB = [w // 128 for w in blocks]
    GBASE = [OFF[c] // 128 for c in range(NSIG)]
    NT = SP // 128
    hst, hsn, w1r, qvt, masku, ctx_out = (
        aps["hst"], aps["hsn"], aps["w1r"],
        aps["qvt"], aps["masku"], aps["ctx"],
    )

    def pool(name, bufs, space="SBUF"):
        return ctx.enter_context(tc.tile_pool(name=name, bufs=bufs, space=space))

    big = SP > 1152  # rare dense-mask case: trade overlap for SBUF headroom
    p_w1 = pool("w1", 1)
    p_hst = pool("hst", 2)     # half-batch tiles, 2 batches in flight
    p_hsn = pool("hsn", 1 if SP > 1280 else 2)
    p_small = pool("small", 1)
    p_tanh = pool("tanh", 3)
    p_acc = pool("acc", 6)
    p_em = pool("em", 3)
    p_ctxs = pool("ctxs", 2)
    p_attZ = pool("attZ", 2)
    p_sc = pool("sc", 4)
    p_mx = pool("mx", 8)

    pp_pre = pool("ppre", 5, space="PSUM")
    pp_misc = pool("pmisc", 1, space="PSUM")
    pp_ctx = pool("pctx", 2, space="PSUM")

    # ---------------- setup DMAs ----------------
    # Host relays every big operand so one DMA moves 9-18KB per partition
    # (fat descriptors ~300GB/s; the naive per-row 2KB descriptors cap a
    # queue row at ~60-100GB/s).  hst is split in two half-batch tiles so
    # block (0,0)'s first k-chunks gate on only half the bytes.
    MA = 4  # phase-A m-count for block (0,0): phase A gates on w1[m0..m3]
            # (1MB on sync) while w1[m4..m7] stream on scalar for phase B
    w1_t = [
        p_w1.tile([128, H], FP16, tag=f"w1_{m}", name=f"w1_{m}", bufs=1)
        for m in range(HK)
    ]
    hstb_t = {}

    HA = HK // 2

    def hst_dma(b, eng=None):
        """Fat half-batch transfers (steady-state batches), split over two
        queue rows (each row gets ~1/3 of aggregate DMA bandwidth)."""
        ta = p_hst.tile([128, HA * SP], FP16, tag="hstr_a", name=f"hst_{b}_a")
        tb = p_hst.tile([128, (HK - HA) * SP], FP16, tag="hstr_b", name=f"hst_{b}_b")
        (eng or nc.gpsimd).dma_start(ta[:], hst[b, :, :HA * SP])
        (eng or nc.scalar).dma_start(tb[:], hst[b, :, HA * SP:])
        hstb_t[b] = [
            (ta[:, SP * k:SP * (k + 1)] if k < HA
             else tb[:, SP * (k - HA):SP * (k - HA + 1)])
            for k in range(HK)
        ]

    hsn_t = {}

    def hsn_dma(b, eng=None):
        h = p_hsn.tile([128, NT * H], FP16, tag="hsnr", name=f"hsn_{b}")
        (eng or nc.sync).dma_start(h[:], hsn[b, :, :])
        hsn_t[b] = [h[:, H * t:H * (t + 1)] for t in range(NT)]

    # warmup source: memset garbage, no DMA dependency.  On DVE -- the
    # gpsimd engine queue must start with its dma_starts (SWDGE) or the
    # first gpsimd-row DMA lands several us late.
    wsrc = p_small.tile([128, 128], FP16, tag="wsrc")
    nc.vector.memset(wsrc[:], 0.5)
    ident = p_small.tile([1, 1], F32, tag="ident")
    nc.vector.memset(ident[:], 1.0)
    ones_h = p_small.tile([128, 1], FP16, tag="ones_h")
    nc.vector.memset(ones_h[:], 1.0)

    # Head-priority DMAs.  Block (0,0) phase A consumes w1[m] x hst0[k]
    # progressively, so both are per-chunk tiles emitted in consumption
    # order, interleaved across the three queue rows (each row gets ~1/3 of
    # the ~300GB/s aggregate while all are busy).
    def w1_dma(m):
        nc.sync.dma_start(w1_t[m][:], w1r[:, H * m:H * (m + 1)])

    if big:
        w1_dma(0)
        hst_dma(0)  # fat halves on gpsimd+scalar; phase A paces off arrival
        for m in range(1, HK):
            w1_dma(m)
    else:
        tiles0 = [
            p_hst.tile([128, SP], FP16, tag=f"h0k{k}", name=f"hst_0_{k}", bufs=1)
            for k in range(HK)
        ]
        hstb_t[0] = [t[:] for t in tiles0]

        def hst0_dma(k, eng):
            eng.dma_start(tiles0[k][:], hst[0, :, SP * k:SP * (k + 1)])

        # sync: w1 m0-m3 (phase A) | scalar: chunks k0,k1,k2,k5 + smalls +
        # w1 m4-m7 (phase B, just-in-time) | gpsimd: chunks k3,k4,k6,k7
        w1_dma(0)
        hst0_dma(0, nc.scalar)
        hst0_dma(3, nc.gpsimd)
        w1_dma(1)
        hst0_dma(1, nc.scalar)
        hst0_dma(4, nc.gpsimd)
        w1_dma(2)
        hst0_dma(2, nc.scalar)
        hst0_dma(6, nc.gpsimd)
        w1_dma(3)
        hst0_dma(5, nc.scalar)
        hst0_dma(7, nc.gpsimd)

    # qvt = [qt | vt] precomputed on host (q = W2 @ hidden + b1 + b2 is only
    # 0.004% of total FLOPs; computing it host-side removes the 2MB W2 DMA
    # from the critical window and the whole batch-0 activation deferral)
    qvt_sb = p_small.tile([128, BL * HK + HK], F32, tag="qvt")
    nc.scalar.dma_start(qvt_sb[:], qvt[:])
    qt_sb = qvt_sb[:, 0:BL * HK]
    vt_sb = qvt_sb[:, BL * HK:BL * HK + HK]
    if use_mask:
        mask_all = p_small.tile([1, BL * SP], U8, tag="mask")
        nc.scalar.dma_start(mask_all[:], masku[:])
    if not big:
        for m in range(MA, HK):
            nc.scalar.dma_start(w1_t[m][:], w1r[:, H * m:H * (m + 1)])
    hsn_dma(0, eng=nc.sync)

    # PE clock warmup on garbage: dense N=128 matmuls until HAM hits K=8/8
    warm = pp_misc.tile([128, 128], F32, tag="ptr", name="warm")
    for _ in range(45):
        nc.tensor.matmul(warm[:], lhsT=wsrc[:], rhs=wsrc[:], start=True, stop=True)

    # ------------- pending-closure FIFO -------------
    pending = []

    def flush_one():
        if pending:
            pending.pop(0)()

    acc_t = {}

    def _act_chain_m(b, c, m, src, W):
        th = p_tanh.tile([128, W], F32, tag="tanh", name=f"th_{b}_{c}_{m}")
        nc.scalar.activation(
            th[:], src, AF.Tanh,
            bias=qt_sb[:, BL * m + b:BL * m + b + 1], scale=1.0,
        )
        if m == 0:
            acc = p_acc.tile([128, W], F32, tag="acc", name=f"acc_{b}_{c}")
            acc_t[(b, c)] = acc
            nc.vector.tensor_scalar_mul(acc[:], th[:], vt_sb[:, 0:1])
        elif m < HK - 1:
            acc = acc_t[(b, c)]
            nc.vector.scalar_tensor_tensor(
                acc[:], th[:], vt_sb[:, m:m + 1], acc[:], op0=OP.mult, op1=OP.add,
            )
        else:
            acc = acc_t.pop((b, c))
            acc_r = p_acc.tile([128, W], FP16, tag="accr", name=f"accr_{b}_{c}")
            acc_t[(b, c)] = acc_r
            nc.vector.scalar_tensor_tensor(
                acc_r[:], th[:], vt_sb[:, m:m + 1], acc[:], op0=OP.mult, op1=OP.add,
            )

    def p1_block00():
        """Block (0,0), DMA-paced: k-outer over 5 PSUM banks so the PE
        consumes hst chunks at arrival pace with no >1us idle windows
        (idle windows re-throttle HAM to 1.2GHz), then k-inner for the rest.
        All 8 pre tiles are DVE-evacuated to SBUF so the ppre banks recycle at
        DVE pace instead of waiting out the 8-tanh ACT burst at block end
        (which would stall (0,1)'s first m-groups ~5us)."""
        W, off = blocks[0], OFF[0]
        hstb = hstb_t[0]
        pre0 = p_small.tile([128, HK * W], F32, tag="pre0")
        pp = [
            pp_pre.tile([128, W], F32, tag="ppre", name=f"pp0_{m}")
            for m in range(MA)
        ]
        # consume chunks in DMA-arrival order (scalar row: k0,k1,k2,k5
        # interleaving with gpsimd row: k3,k4,k6,k7); the PSUM accumulation
        # order over k is irrelevant
        korder = [0, 3, 1, 4, 2, 6, 5, 7] if not big else list(range(HK))
        for ki, k in enumerate(korder):
            if ki:
                # filler matmuls on garbage: the k-step is DMA-arrival-bound,
                # so these are free and keep HAM from re-throttling the PE
                # during the chunk gaps (early steps wait longest)
                for _ in range(12):
                    nc.tensor.matmul(warm[:], lhsT=wsrc[:], rhs=wsrc[:],
                                     start=True, stop=True)
            for m in range(MA):
                nc.tensor.matmul(
                    pp[m][:],
                    lhsT=w1_t[m][:, 128 * k:128 * (k + 1)],
                    rhs=hstb[k][:, off:off + W],
                    start=(ki == 0), stop=(ki == HK - 1),
                )
        for m in range(MA):
            nc.vector.tensor_copy(pre0[:, W * m:W * (m + 1)], pp[m][:])
        for m in range(MA, HK):
            ppre = pp_pre.tile([128, W], F32, tag="ppre", name=f"ppre_0_0_{m}")
            for k in range(HK):
                nc.tensor.matmul(
                    ppre[:],
                    lhsT=w1_t[m][:, 128 * k:128 * (k + 1)],
                    rhs=hstb[k][:, off:off + W],
                    start=(k == 0), stop=(k == HK - 1),
                )
            nc.vector.tensor_copy(pre0[:, W * m:W * (m + 1)], ppre[:])
        for m in range(HK):
            _act_chain_m(0, 0, m, pre0[:, W * m:W * (m + 1)], W)

    def p1_block(b, c, all_slots=False):
        W, off = blocks[c], OFF[c]
        if c == min(1, NSIG - 1) and b + 1 < BL:
            hst_dma(b + 1)
        slots = set(range(HK)) if all_slots else {1, 3, 5, 7}
        if NSIG == 2 and not all_slots:
            slots |= {2, 6}
        hstb = hstb_t[b]
        for m in range(HK):
            if m in slots:
                flush_one()
            ppre = pp_pre.tile([128, W], F32, tag="ppre", name=f"ppre_{b}_{c}_{m}")
            for k in range(HK):
                nc.tensor.matmul(
                    ppre[:],
                    lhsT=w1_t[m][:, 128 * k:128 * (k + 1)],
                    rhs=hstb[k][:, off:off + W],
                    start=(k == 0), stop=(k == HK - 1),
                )
            _act_chain_m(b, c, m, ppre[:], W)

    # ------------- per-block tail: tp -> tmid -> post -------------
    mx_t, zf_t, em_t, attZ_t, csum_t = {}, {}, {}, {}, {}

    def tail_tp(b, c):
        W, off = blocks[c], OFF[c]
        acc_r = acc_t.pop((b, c))
        pen = pp_misc.tile([1, W], F32, tag="ptr", name=f"pen_{b}_{c}")
        nc.tensor.matmul(pen[:], lhsT=ones_h[:], rhs=acc_r[:], start=True, stop=True)
        if c == 0:
            mx_t[b] = p_mx.tile([1, NSIG], F32, tag="mx", name=f"mx{b}")
            zf_t[b] = p_mx.tile([1, NSIG], F32, tag="zf", name=f"zf{b}")
        em = p_em.tile([1, W], F32, tag="em", name=f"em_{b}_{c}")
        if use_mask:
            nc.vector.scalar_tensor_tensor(
                em[:], mask_all[:, b * SP + off:b * SP + off + W], -1e10, pen[:],
                op0=OP.mult, op1=OP.add,
            )
            src = em
        else:
            # padded hst columns hold z = W1^-1(-q - K sign(v)), so their
            # energy is ~ -sum|v| and softmax zeroes them without a mask
            src = pen
        em_t[(b, c)] = em
        nc.vector.reduce_max(mx_t[b][:, c:c + 1], src[:], axis=AX.X)
        nmx = p_sc.tile([1, 1], F32, tag="nmx", name=f"nmx_{b}_{c}")
        nc.vector.tensor_scalar_mul(nmx[:], mx_t[b][:, c:c + 1], -1.0)
        nc.scalar.activation(
            em[:], src[:], AF.Exp, bias=nmx[:], scale=1.0,
            accum_out=zf_t[b][:, c:c + 1],
        )

    def tail_tmid(b, c):
        T = TPB[c]
        em = em_t.pop((b, c))
        if c == 0:
            attZ = p_attZ.tile([128, NSIG * NT], FP16, tag="attZ", name=f"attZ{b}")
            nc.vector.memset(attZ[:], 0.0)
            attZ_t[b] = attZ
        attZ = attZ_t[b]
        ptr = pp_misc.tile([128, T], F32, tag="ptr", name=f"ptr_{b}_{c}")
        for j in range(T):
            nc.tensor.transpose(ptr[:, j:j + 1], em[:, 128 * j:128 * (j + 1)], ident[:])
        for j in range(T):
            t = GBASE[c] + j
            nc.vector.tensor_copy(attZ[:, NSIG * t + c:NSIG * t + c + 1], ptr[:, j:j + 1])

    def tail_post(b, c):
        T = TPB[c]
        attZ = attZ_t[b]
        if c == 0:
            csum_t[b] = [
                pp_ctx.tile([NSIG, 512], F32, tag="pctx", name=f"csum_{b}_{n}")
                for n in range(2)
            ]
        csum = csum_t[b]
        tiles = hsn_t[b]
        for j in range(T):
            t = GBASE[c] + j
            for n in range(2):
                nc.tensor.matmul(
                    csum[n][:],
                    lhsT=attZ[:, NSIG * t:NSIG * (t + 1)],
                    rhs=tiles[t][:, 512 * n:512 * (n + 1)],
                    start=(c == 0 and j == 0), stop=(c == NSIG - 1 and j == T - 1),
                )

    # ------------- merge: merge_a (DVE chain + evac) / merge_b (matmuls) ----
    wrz_t, csr_t = {}, {}

    def merge_a(b):
        mx = mx_t.pop(b)
        zf = zf_t.pop(b)
        attZ_t.pop(b)
        csum = csum_t.pop(b)
        negM = p_sc.tile([1, 1], F32, tag="negM", name=f"negM{b}")
        nc.vector.reduce_max(negM[:], mx[:], axis=AX.X, negate=True)
        wt = p_mx.tile([1, NSIG], F32, tag="wt", name=f"wt{b}")
        nc.scalar.activation(wt[:], mx[:], AF.Exp, bias=negM[:], scale=1.0)
        zw = p_mx.tile([1, NSIG], F32, tag="zw", name=f"zw{b}")
        nc.vector.tensor_mul(zw[:], wt[:], zf[:])
        zs = p_sc.tile([1, 1], F32, tag="zs", name=f"zs{b}")
        nc.vector.reduce_sum(zs[:], zw[:], axis=AX.X)
        rz = p_sc.tile([1, 1], F32, tag="rz", name=f"rz{b}")
        nc.vector.reciprocal(rz[:], zs[:])
        wrz = p_mx.tile([1, NSIG], F32, tag="wrz", name=f"wrz{b}")
        nc.vector.tensor_scalar_mul(wrz[:], wt[:], rz[:])
        ptrw = pp_misc.tile([NSIG, 1], F32, tag="ptr", name=f"ptrw{b}")
        nc.tensor.transpose(ptrw[:], wrz[:], ident[:])
        wrz_h = p_mx.tile([NSIG, 1], FP16, tag="wrz_h", name=f"wrz_h{b}")
        nc.vector.tensor_copy(wrz_h[:], ptrw[:])
        wrz_t[b] = wrz_h
        csr = p_ctxs.tile([NSIG, H], FP16, tag="csr", name=f"csr{b}")
        for n in range(2):
            nc.vector.tensor_copy(csr[:, 512 * n:512 * (n + 1)], csum[n][:])
        csr_t[b] = csr

    def merge_b(b):
        wrz_h = wrz_t.pop(b)
        csr = csr_t.pop(b)
        cs = p_ctxs.tile([1, H], F32, tag="cs", name=f"cs{b}")
        # both matmuls back-to-back (mo1 borrows the pctx bank csum just
        # freed), then the two evacuations run on DVE and ACT in parallel
        mo0 = pp_misc.tile([1, 512], F32, tag="ptr", name=f"mo_{b}_0")
        mo1 = pp_ctx.tile([1, 512], F32, tag="pctx", name=f"mo_{b}_1")
        nc.tensor.matmul(mo0[:], lhsT=wrz_h[:], rhs=csr[:, 0:512],
                         start=True, stop=True)
        nc.tensor.matmul(mo1[:], lhsT=wrz_h[:], rhs=csr[:, 512:1024],
                         start=True, stop=True)
        nc.vector.tensor_copy(cs[:, 0:512], mo0[:])
        nc.scalar.activation(cs[:, 512:1024], mo1[:], AF.Copy)
        nc.sync.dma_start(ctx_out[b:b + 1, 0:512], cs[:, 0:512])
        nc.sync.dma_start(ctx_out[b:b + 1, 512:1024], cs[:, 512:1024])

    def append_tail(b, c):
        pending.append(lambda: tail_tp(b, c))
        pending.append(lambda: tail_tmid(b, c))
        pending.append(lambda: tail_post(b, c))
        if c == 0 and b + 1 < BL:
            # hsn prefetch goes through the FIFO so it is traced AFTER the
            # previous batch's context matmuls released the hsn slots.
            pending.append(lambda: hsn_dma(b + 1))

    # ------------- schedule -------------
    p1_block00()
    if NSIG == 1:
        hst_dma(1)
    append_tail(0, 0)
    for c in range(1, NSIG):
        p1_block(0, c)
        append_tail(0, c)
    pending.append(lambda: merge_a(0))
    pending.append(lambda: merge_b(0))
    for b in range(1, BL):
        for c in range(NSIG):
            p1_block(b, c, all_slots=(b == BL - 1))
            append_tail(b, c)
        pending.append(lambda bb=b: merge_a(bb))
        pending.append(lambda bb=b: merge_b(bb))
    while pending:
        flush_one()


def build_program(blocks, use_mask=False):
    blocks = tuple(blocks)
    key = ("nc", blocks, use_mask)
    if key in _CACHE:
        return _CACHE[key]
    SP = sum(blocks)
    nc = bacc.Bacc("TRN2", target_bir_lowering=False, debug=False, enable_asserts=False)
    NT = SP // 128
    aps = {
        "hst": nc.dram_tensor("hst", (BL, 128, HK * SP), FP16, kind="ExternalInput").ap(),
        "hsn": nc.dram_tensor("hsn", (BL, 128, NT * H), FP16, kind="ExternalInput").ap(),
        "w1r": nc.dram_tensor("w1r", (128, HK * H), FP16, kind="ExternalInput").ap(),
        "qvt": nc.dram_tensor("qvt", (128, BL * HK + HK), F32, kind="ExternalInput").ap(),
        "masku": nc.dram_tensor("masku", (1, BL * SP), U8, kind="ExternalInput").ap(),
        "ctx": nc.dram_tensor("ctx", (BL, H), F32, kind="ExternalOutput").ap(),
    }
    with tile.TileContext(nc) as tc:
        with ExitStack() as stack:
            aps["ctx_stack"] = stack
            _emit(tc, aps, blocks, use_mask)
    nc.compile()
    _CACHE[key] = nc
    return nc


def prep_in_maps(inputs, blocks):
    SP = sum(blocks)
    hidden = np.ascontiguousarray(np.asarray(inputs["hidden"], dtype=np.float32))
    hs = np.asarray(inputs["hidden_sequence"], dtype=np.float32)
    masks = np.asarray(inputs["input_masks"]).astype(bool)
    w1t = np.asarray(inputs["W1"], dtype=np.float32).T  # (hin, hout)
    # m-major relayout: w1r[:, 1024*m + 128*k : +128] = W1T[128k:128(k+1), 128m:128(m+1)]
    w1r = np.ascontiguousarray(
        w1t.reshape(HK, 128, HK, 128).transpose(1, 2, 0, 3).reshape(128, HK * H)
        .astype(np.float16)
    )
    b1 = np.asarray(inputs["b1"], dtype=np.float32)
    b2 = np.asarray(inputs["b2"], dtype=np.float32)
    v = np.asarray(inputs["v"], dtype=np.float32)
    # q = W2 @ hidden + b1 + b2 on host (67 MFLOP, 0.004% of the kernel's
    # FLOPs) -- removes the 2MB W2 DMA + 4.3us of PE work from the device
    w2 = np.asarray(inputs["W2"], dtype=np.float32)
    q_all = hidden[0] @ w2.T + b1 + b2          # (B, H)
    vt = v.reshape(HK, 128).T                   # (128, HK)
    # mask-free padding: z_b = W1^-1(-q_b - K sign(v)) makes every padded
    # column's energy ~= -sum|v| so softmax zeroes it without a device mask
    # (A mask-free variant -- filling padded columns with z = W1^-1(-q - K
    # sign(v)) so softmax zeroes them naturally -- was measured ~8us SLOWER:
    # reduce_max/exp reading the pen PSUM directly is slower than the DVE
    # mask-stt that moves it to SBUF first.  Keep the mask path.)
    use_mask = True
    z_all = None
    in_maps = []
    for ci in range(NCORES):
        g = slice(BL * ci, BL * (ci + 1))
        # qt[p, BL*m + b] = q_all[g][b, 128m + p]
        qt = q_all[g].reshape(BL, HK, 128).transpose(2, 1, 0).reshape(128, HK * BL)
        qvt = np.ascontiguousarray(
            np.concatenate([qt, vt], axis=1).astype(np.float32))
        NT = SP // 128
        hst_p = np.zeros((BL, H, SP), dtype=np.float16)
        hsn_p = np.zeros((BL, SP, H), dtype=np.float16)
        maskp = np.ones((BL, SP), dtype=np.uint8)  # 1 = padded (masked out)
        for b in range(BL):
            gb = BL * ci + b
            idx = np.nonzero(masks[:, gb])[0]
            n = idx.shape[0]
            sel = hs[idx, gb, :].astype(np.float16)  # (n, H)
            hst_p[b, :, :n] = sel.T
            if not use_mask:
                hst_p[b, :, n:] = z_all[:, gb:gb + 1].astype(np.float16)
            hsn_p[b, :n, :] = sel
            maskp[b, :n] = 0
        # fat-descriptor relayouts: (BL,128,HK*SP) and (BL,128,NT*H)
        hst_r = np.ascontiguousarray(
            hst_p.reshape(BL, HK, 128, SP).transpose(0, 2, 1, 3)
            .reshape(BL, 128, HK * SP))
        hsn_r = np.ascontiguousarray(
            hsn_p.reshape(BL, NT, 128, H).transpose(0, 2, 1, 3)
            .reshape(BL, 128, NT * H))
        in_maps.append({
            "hst": hst_r,
            "hsn": hsn_r,
            "w1r": w1r,
            "qvt": qvt,
            "masku": maskp.reshape(1, BL * SP),
        })
    return in_maps, use_mask


def kernel(**inputs):
    blocks = pick_blocks(inputs["input_masks"])
    in_maps, use_mask = prep_in_maps(inputs, blocks)
    nc = build_program(blocks, use_mask)
    res = bass_utils.run_bass_kernel_spmd(nc, in_maps, list(range(NCORES)))
    out = np.concatenate([res.results[i]["ctx"] for i in range(NCORES)], axis=0)
    return out[None].astype(np.float32)


if __name__ == "__main__":
    build_program((512, 512, 128))
    print("program built OK")
